# revision 12
# baseline (speedup 1.0000x reference)
"""DHASPI level-loss kernel v2 for 8 Trainium2 NeuronCores.

Data-parallel over the fused B*C row axis (64 x-rows + 64 y-rows per core in
the 128 SBUF partitions). The work is spread across all four engine queues:

- Pool (gpsimd SWDGE): casting DMAs f32 HBM -> fp16 SBUF (half the modeled
  DMA cost) for "H" chunks, plus in-place fold-assists on late chunks.
- SP (sync HWDGE): f32 DMAs for "F" chunks, running concurrently with Pool's.
- ACT: squares (f32 or fp16 in -> fp16 out) + the final Ln.
- DVE: fp16 squares of some H chunks (2x perf mode), block-sum fold trees
  (in-place halving adds at 2x + a final 60-wide tensor_reduce), epilogue.

Block sums: each 960-sample block is folded in place inside its square tile
960->480->240->120->60, then one tensor_reduce produces the per-block sums.
All bulk tiles are fp16 (rel err ~5e-4, far inside the 2e-2 gate); the gated
loudness math is f32 with the 1/FRAME scaling folded into the log constants.

The last 960 samples of each row feed no analysis frame and are never loaded.
"""

import math

import numpy as np

import concourse.bass as bass
from concourse import mybir
from concourse.bass_utils import run_bass_kernel_spmd

B, C, T = 16, 32, 192000
N_CORES = 8
ROWS = B * C
RPC = ROWS // N_CORES

FRAME = 9600
SHIFT = 2880
BLK = 960
NBLK_USED = 199          # block 199 (samples 191040..192000) feeds no frame
NFRM = (T - FRAME) // SHIFT + 1  # 64

EPS = 1e-8
ALPHA = 1e-4
GAMMA_A = -70.0
# zs = 9600 * z domain constants
TA_ZS = float(FRAME * (10.0 ** ((GAMMA_A + 0.691) / 10.0) - EPS))
TR_OFF_ZS = float(-0.9 * FRAME * EPS)
EPS_LN = float(FRAME * EPS)
LN10_INV10 = float(10.0 / math.log(10.0))
C_LUFS = float(-0.691 - 10.0 * math.log10(FRAME))

F32 = mybir.dt.float32
F16 = mybir.dt.float16

# ---------------------------------------------------------------- schedule --
# Chunk = contiguous run of 960-sample blocks (all 128 rows).
# kind 'H': fp16 via Pool cast-DMA | 'F': f32 via SP DMA
# sq 'A': ACT square | 'D': DVE square (H only)
# assist: 0 = DVE folds alone; 2/3 = Pool folds in place to 240/120 first


def _schedule():
    chunks = []

    def add(kind, nblk, sq, assist=0):
        chunks.append(
            {"kind": kind, "nblk": nblk, "sq": sq, "assist": assist, "id": len(chunks)}
        )

    # warm-up slices (alternating DVE/ACT squares) so compute starts early
    for k in range(5):
        add("H", 2, "D" if k % 2 == 0 else "A")
    for k in range(14):         # h1..h14; every 3rd is Pool-squared
        add("H", 5, "P" if k % 3 == 2 else "A")
    # late H chunks at 4800, Pool fold-assists to 120-wide; the last 3 are
    # fully Pool-owned (square+folds back-to-back on Pool's queue)
    for k in range(8):
        add("H", 5, "D", assist=3)
    # F region: remaining 79 blocks: 14 x 4800 + tail 4 x 1920 + 960
    for _ in range(14):
        add("F", 5, "A")
    for _ in range(4):
        add("F", 2, "A")
    add("F", 1, "A")

    off = 0
    for ch in chunks:
        if ch["kind"] == "H":
            ch["blk0"] = off
            off += ch["nblk"]
    for ch in chunks:
        if ch["kind"] == "F":
            ch["blk0"] = off
            off += ch["nblk"]
    assert off == NBLK_USED, off
    return chunks


CHUNKS = _schedule()
H_IDS = [c["id"] for c in CHUNKS if c["kind"] == "H"]
F_IDS = [c["id"] for c in CHUNKS if c["kind"] == "F"]
POOL_DMA_ORDER = H_IDS
SP_DMA_ORDER = F_IDS
ASSIST_IDS = [c["id"] for c in CHUNKS if c["assist"]]

# estimated DMA completion times (us-ish units) for ordering heuristics
_t_pool = 0.0
_t_sp = 0.0
READY = {}
for _ch in CHUNKS:
    if _ch["kind"] == "H":
        _t_pool += 7.402 * _ch["nblk"] / 10.0
        READY[_ch["id"]] = _t_pool + 2.9
    else:
        _t_sp += 14.805 * _ch["nblk"] / 10.0
        READY[_ch["id"]] = _t_sp + 2.8

ACT_SQ_ORDER = sorted(
    (c["id"] for c in CHUNKS if c["sq"] == "A"), key=lambda i: READY[i]
)
# DVE-squared chunks in arrival order
D_SEQ = sorted((c["id"] for c in CHUNKS if c["sq"] == "D"), key=lambda i: READY[i])
D_POS = {i: n for n, i in enumerate(D_SEQ)}


def _dve_order():
    # 'sq' jobs at arrival; assisted tree of D-chunk k goes after the square
    # of D-chunk k+2 so every Pool<->DVE wait points backwards in both queues
    jobs = []
    for ch in CHUNKS:
        i = ch["id"]
        if ch["sq"] == "D":
            jobs.append((READY[i], 0, ("sq", i)))
            t = READY[i] + (6.0 if ch["assist"] else 0.5)
            if ch["assist"]:
                k = D_POS[i]
                if k + 2 < len(D_SEQ):
                    t = max(t, READY[D_SEQ[k + 2]] + 0.2)
                else:
                    t = max(t, READY[D_SEQ[-1]] + 0.2 + 0.01 * k)
            jobs.append((t, 1, ("tree", i)))
        elif ch["sq"] == "P":
            jobs.append((READY[i] + (10.0 if ch["assist"] else 8.5), 1, ("tree", i)))
        else:
            jobs.append((READY[i] + 8.3, 1, ("tree", i)))
    jobs.sort()
    return [j for _, _, j in jobs]


DVE_ORDER = _dve_order()
DVE_SQ_SEQ = [i for j, i in DVE_ORDER if j == "sq"]


def _pool_prog():
    # assist for D-chunk at D_SEQ position k rides right after the DMA of
    # D-chunk k+2 (whose square will wait for this assist's chunk tree);
    # Pool squares ('P') ride one DMA after their own
    after_dma = {}
    tail = []
    for a in ASSIST_IDS:
        if a not in D_POS:
            continue  # P-chunk assists are emitted with their psq below
        k = D_POS[a]
        if k + 2 < len(D_SEQ):
            after_dma.setdefault(D_SEQ[k + 2], []).append(("assist", a))
        else:
            tail.append(("assist", a))
    for n, i in enumerate(H_IDS):
        if CHUNKS[i]["sq"] == "P":
            jobs = [("psq", i)]
            if CHUNKS[i]["assist"]:
                jobs.append(("assist", i))
            if n + 1 < len(H_IDS):
                after_dma.setdefault(H_IDS[n + 1], []).extend(jobs)
            else:
                tail.extend(jobs)
    prog = []
    for i in H_IDS:
        prog.append(("dma", i))
        prog.extend(after_dma.get(i, []))
    prog.extend(tail)
    return prog


POOL_PROG = _pool_prog()

NXH = 6   # fp16 input slots [128, 4800]; H squares+folds run in place here
NXF = 6   # f32 input slots [128, 4800]
NSQA = 3  # fp16 square slots for F chunks [128, 4800]


def _build_program() -> bass.Bass:
    nc = bass.Bass("TRN2", target_bir_lowering=False, debug=False)
    AF = mybir.ActivationFunctionType
    ALU = mybir.AluOpType
    AX = mybir.AxisListType

    xy = nc.dram_tensor("xy", [128, T], F32, kind="ExternalInput").ap()
    out = nc.dram_tensor("lufs", [128, 1], F32, kind="ExternalOutput").ap()

    xh = [nc.alloc_sbuf_tensor(f"xh{i}", [128, FRAME // 2], F16).ap() for i in range(NXH)]
    xf = [nc.alloc_sbuf_tensor(f"xf{i}", [128, FRAME // 2], F32).ap() for i in range(NXF)]
    sqA = [nc.alloc_sbuf_tensor(f"sqA{i}", [128, FRAME // 2], F16).ap() for i in range(NSQA)]
    bs = nc.alloc_sbuf_tensor("bs", [128, 200], F16).ap()
    zs = nc.alloc_sbuf_tensor("zs", [128, NFRM], F32).ap()
    ga = nc.alloc_sbuf_tensor("ga", [128, NFRM], F32).ap()
    ma = nc.alloc_sbuf_tensor("ma", [128, NFRM], F32).ap()
    gar = nc.alloc_sbuf_tensor("gar", [128, NFRM], F32).ap()
    sc = nc.alloc_sbuf_tensor("sc", [128, 12], F32).ap()
    eps_t = nc.alloc_sbuf_tensor("eps_t", [128, 1], F32).ap()

    numa, dena, rca, zavea = sc[:, 0:1], sc[:, 1:2], sc[:, 2:3], sc[:, 3:4]
    thr, denar, numar, rcar = sc[:, 4:5], sc[:, 5:6], sc[:, 6:7], sc[:, 7:8]
    zavear, lnz, lufs_t = sc[:, 8:9], sc[:, 9:10], sc[:, 10:11]

    # ---- bookkeeping ----------------------------------------------------
    info = {c["id"]: dict(c) for c in CHUNKS}
    for n, i in enumerate(POOL_DMA_ORDER):
        info[i]["dma_ord"] = n
        info[i]["xslot"] = n % NXH
    for n, i in enumerate(SP_DMA_ORDER):
        info[i]["dma_ord"] = n
        info[i]["xslot"] = n % NXF
    for n, i in enumerate(ACT_SQ_ORDER):
        info[i]["sq_sem"] = "qa"
        info[i]["sq_ord"] = n
    for n, i in enumerate(DVE_SQ_SEQ):
        info[i]["sq_sem"] = "qd"
        info[i]["sq_ord"] = n
    for n, i in enumerate([c["id"] for c in CHUNKS if c["sq"] == "P"]):
        info[i]["sq_sem"] = "qp"
        info[i]["sq_ord"] = n
    F_SQA = [i for i in ACT_SQ_ORDER if info[i]["kind"] == "F"]
    for n, i in enumerate(F_SQA):
        info[i]["sq_slot"] = n % NSQA
        info[i]["fsq_ord"] = n
    _pool_assists = [i for k, i in POOL_PROG if k == "assist"]
    assert sorted(_pool_assists) == sorted(ASSIST_IDS)
    for n, i in enumerate(_pool_assists):
        info[i]["as_ord"] = n
    n = 0
    for job, i in DVE_ORDER:
        if job == "tree":
            info[i]["tree_ord"] = n
            n += 1
    n_trees = n

    def sq_tile(ch):
        # H chunks square and fold in place inside their xh slot
        if ch["kind"] == "H":
            return xh[ch["xslot"]]
        return sqA[ch["sq_slot"]]

    def bs_slice(ch):
        return bs[:, ch["blk0"] : ch["blk0"] + ch["nblk"]]

    def bview(ch, w):
        """[128, nblk, w] view of the chunk's square tile (block stride 960)."""
        t = sq_tile(ch)
        base = t[:, 0:1]
        return type(base)(
            tensor=base.tensor,
            offset=base.offset,
            ap=[list(base.ap[0]), [BLK, ch["nblk"]], [1, w]],
        )

    with (
        nc.Block() as block,
        nc.semaphore("s_pl") as s_pl,
        nc.semaphore("s_sp") as s_sp,
        nc.semaphore("s_qa") as s_qa,
        nc.semaphore("s_qd") as s_qd,
        nc.semaphore("s_tp") as s_tp,
        nc.semaphore("s_qp") as s_qp,
        nc.semaphore("s_tv") as s_tv,
        nc.semaphore("s_ln") as s_ln,
        nc.semaphore("s_out") as s_out,
        nc.allow_low_precision("fp16 block sums; rel err ~5e-4 vs 2e-2 gate"),
    ):
        sems = {"qa": s_qa, "qd": s_qd, "qp": s_qp}

        # ------------------------------------------------------------ Pool --
        @block.gpsimd
        def _(g):
            ndma = 0
            for kind_, i in POOL_PROG:
                ch = info[i]
                nb = ch["nblk"]
                if kind_ == "dma":
                    if ndma >= NXH:
                        prev = info[POOL_DMA_ORDER[ndma - NXH]]
                        g.wait_ge(s_tv, prev["tree_ord"] + 1)
                    b0 = ch["blk0"]
                    g.dma_start(
                        out=xh[ch["xslot"]][:, 0 : nb * BLK],
                        in_=xy[:, b0 * BLK : (b0 + nb) * BLK],
                    ).then_inc(s_pl, 16)
                    ndma += 1
                    continue
                if kind_ == "psq":
                    g.wait_ge(s_pl, 16 * (ch["dma_ord"] + 1))
                    t = xh[ch["xslot"]][:, 0 : nb * BLK]
                    g.tensor_tensor(t, t, t, op=ALU.mult)
                    g.drain().then_inc(s_qp, 1)
                    continue
                # in-place fold assist on the chunk's square tile
                g.wait_ge(sems[ch["sq_sem"]], ch["sq_ord"] + 1)
                g.tensor_tensor(
                    bview(ch, 480), bview(ch, 480),
                    bview(ch, 960)[:, :, 480:960], op=ALU.add,
                )
                g.drain()
                g.tensor_tensor(
                    bview(ch, 240), bview(ch, 240),
                    bview(ch, 480)[:, :, 240:480], op=ALU.add,
                )
                if ch["assist"] >= 3:
                    g.drain()
                    g.tensor_tensor(
                        bview(ch, 120), bview(ch, 120),
                        bview(ch, 240)[:, :, 120:240], op=ALU.add,
                    )
                g.drain().then_inc(s_tp, 1)

        # -------------------------------------------------------------- SP --
        @block.sync
        def _(sync):
            for n, i in enumerate(SP_DMA_ORDER):
                ch = info[i]
                if n >= NXF:
                    prev = info[SP_DMA_ORDER[n - NXF]]
                    sync.wait_ge(sems[prev["sq_sem"]], prev["sq_ord"] + 1)
                b0, nb = ch["blk0"], ch["nblk"]
                sync.dma_start(
                    out=xf[ch["xslot"]][:, 0 : nb * BLK],
                    in_=xy[:, b0 * BLK : (b0 + nb) * BLK],
                ).then_inc(s_sp, 16)
            sync.wait_ge(s_ln, 2)
            sync.dma_start(out=out, in_=lufs_t).then_inc(s_out, 16)
            sync.wait_ge(s_out, 16)

        # ------------------------------------------------------------- ACT --
        @block.scalar
        def _(scalar):
            for n, i in enumerate(ACT_SQ_ORDER):
                ch = info[i]
                nb = ch["nblk"]
                if ch["kind"] == "H":
                    scalar.wait_ge(s_pl, 16 * (ch["dma_ord"] + 1))
                    src = xh[ch["xslot"]]
                else:
                    scalar.wait_ge(s_sp, 16 * (ch["dma_ord"] + 1))
                    src = xf[ch["xslot"]]
                if ch["kind"] == "F" and ch["fsq_ord"] >= NSQA:
                    prev = info[F_SQA[ch["fsq_ord"] - NSQA]]
                    scalar.wait_ge(s_tv, prev["tree_ord"] + 1)
                scalar.activation(
                    sq_tile(ch)[:, 0 : nb * BLK], src[:, 0 : nb * BLK], AF.Square
                )
                scalar.drain().then_inc(s_qa, 1)
            # warm the Ln table while idle so the final Ln is cheap
            scalar.activation(lnz, eps_t, AF.Ln, bias=eps_t)
            scalar.drain()
            scalar.wait_ge(s_tv, n_trees + 1)
            scalar.activation(lnz, zavear, AF.Ln, bias=eps_t)
            scalar.drain().then_inc(s_ln, 1)

        # ------------------------------------------------------------- DVE --
        @block.vector
        def _(vector):
            ALU_ = ALU
            vector.memset(eps_t, EPS_LN)

            def tree_waits(ch):
                if ch["assist"]:
                    vector.wait_ge(s_tp, ch["as_ord"] + 1)
                    return 240 if ch["assist"] == 2 else 120
                if ch["sq_sem"] == "qa":
                    vector.wait_ge(s_qa, ch["sq_ord"] + 1)
                elif ch["sq_sem"] == "qp":
                    vector.wait_ge(s_qp, ch["sq_ord"] + 1)
                return 960

            def emit_trees(group):
                # interleave the chunks' in-place fold chains so one drain
                # covers a whole stage across the group (same-engine RAW is
                # per-chunk; different chunks touch disjoint tiles)
                ws = [tree_waits(c) for c in group]
                while any(w > 60 for w in ws):
                    for k, c in enumerate(group):
                        if ws[k] > 60:
                            h = ws[k] // 2
                            vector.tensor_tensor(
                                bview(c, h), bview(c, h),
                                bview(c, ws[k])[:, :, h : ws[k]], op=ALU_.add,
                            )
                            ws[k] = h
                    vector.drain()
                for c in group:
                    vector.tensor_reduce(
                        bs_slice(c), bview(c, 60), axis=AX.X, op=ALU_.add
                    )
                vector.drain().then_inc(s_tv, len(group))

            def pairable(a, b):
                # only pair late trees whose data is already banked; early
                # pairing couples waits and stalls upstream producers
                return (
                    READY[a["id"]] > 120.0
                    and READY[b["id"]] > 120.0
                    and abs(READY[a["id"]] - READY[b["id"]]) < 8.0
                )

            pending = None
            for job, i in DVE_ORDER:
                ch = info[i]
                nb = ch["nblk"]
                if job == "sq":
                    if pending is not None:
                        emit_trees([pending])
                        pending = None
                    vector.wait_ge(s_pl, 16 * (ch["dma_ord"] + 1))
                    src = xh[ch["xslot"]][:, 0 : nb * BLK]
                    vector.tensor_tensor(
                        sq_tile(ch)[:, 0 : nb * BLK], src, src, op=ALU_.mult
                    )
                    vector.drain().then_inc(s_qd, 1)
                    continue
                if pending is not None:
                    if pairable(pending, ch):
                        emit_trees([pending, ch])
                        pending = None
                    else:
                        emit_trees([pending])
                        pending = ch
                else:
                    pending = ch
            if pending is not None:
                emit_trees([pending])

            # -------------------------------------------------- epilogue ----
            base = bs[:, 0:1]
            frames_view = type(base)(
                tensor=base.tensor,
                offset=base.offset,
                ap=[list(base.ap[0]), [3, NFRM], [1, FRAME // BLK]],
            )
            vector.tensor_reduce(zs, frames_view, axis=AX.X, op=ALU_.add)
            vector.drain()
            vector.scalar_tensor_tensor(
                out=ga, in0=zs, scalar=TA_ZS, in1=zs, op0=ALU_.is_gt, op1=ALU_.mult
            )
            vector.tensor_scalar(ma, zs, TA_ZS, None, op0=ALU_.is_gt)
            vector.drain()
            vector.reduce_sum(numa, ga, axis=AX.X)
            vector.reduce_sum(dena, ma, axis=AX.X)
            vector.drain()
            vector.tensor_scalar_add(dena, dena, EPS)
            vector.drain()
            vector.reciprocal(rca, dena)
            vector.drain()
            vector.tensor_tensor(zavea, numa, rca, op=ALU_.mult)
            vector.drain()
            vector.tensor_scalar(thr, zavea, 0.1, TR_OFF_ZS, op0=ALU_.mult, op1=ALU_.add)
            vector.drain()
            vector.scalar_tensor_tensor(
                out=gar, in0=zs, scalar=thr, in1=ma, op0=ALU_.is_gt, op1=ALU_.mult
            )
            vector.scalar_tensor_tensor(
                out=ga, in0=zs, scalar=thr, in1=ga, op0=ALU_.is_gt, op1=ALU_.mult
            )
            vector.drain()
            vector.reduce_sum(denar, gar, axis=AX.X)
            vector.reduce_sum(numar, ga, axis=AX.X)
            vector.drain()
            vector.tensor_scalar_add(denar, denar, EPS)
            vector.drain()
            vector.reciprocal(rcar, denar)
            vector.drain()
            vector.tensor_tensor(zavear, numar, rcar, op=ALU_.mult)
            vector.drain().then_inc(s_tv, 1)
            vector.wait_ge(s_ln, 1)
            vector.tensor_scalar(
                lufs_t, lnz, LN10_INV10, C_LUFS, op0=ALU_.mult, op1=ALU_.add
            )
            vector.drain().then_inc(s_ln, 1)

    return nc


def make_in_maps(x_env: np.ndarray, y_env: np.ndarray) -> list[dict[str, np.ndarray]]:
    x = np.asarray(x_env, dtype=np.float32).reshape(ROWS, T)
    y = np.asarray(y_env, dtype=np.float32).reshape(ROWS, T)
    in_maps = []
    for i in range(N_CORES):
        shard = np.concatenate(
            [x[i * RPC : (i + 1) * RPC], y[i * RPC : (i + 1) * RPC]], axis=0
        )
        in_maps.append({"xy": np.ascontiguousarray(shard)})
    return in_maps


def finish(per_core_lufs: list[np.ndarray]) -> np.ndarray:
    total = 0.0
    for lf in per_core_lufs:
        lf = np.asarray(lf).reshape(128).astype(np.float64)
        total += np.maximum(lf[RPC:] - lf[:RPC], 0.0).sum()
    return np.array(ALPHA * total, dtype=np.float32)


def kernel(x_env: np.ndarray, y_env: np.ndarray) -> np.ndarray:
    nc = _build_program()
    in_maps = make_in_maps(x_env, y_env)
    res = run_bass_kernel_spmd(nc, in_maps, core_ids=list(range(N_CORES)))
    return finish([res.results[i]["lufs"] for i in range(N_CORES)])


# revision 13
# speedup vs baseline: 1.0152x; 1.0152x over previous
"""DHASPI level-loss kernel v2 for 8 Trainium2 NeuronCores.

Data-parallel over the fused B*C row axis (64 x-rows + 64 y-rows per core in
the 128 SBUF partitions). The work is spread across all four engine queues:

- Pool (gpsimd SWDGE): casting DMAs f32 HBM -> fp16 SBUF (half the modeled
  DMA cost) for "H" chunks, plus in-place fold-assists on late chunks.
- SP (sync HWDGE): f32 DMAs for "F" chunks, running concurrently with Pool's.
- ACT: squares (f32 or fp16 in -> fp16 out) + the final Ln.
- DVE: fp16 squares of some H chunks (2x perf mode), block-sum fold trees
  (in-place halving adds at 2x + a final 60-wide tensor_reduce), epilogue.

Block sums: each 960-sample block is folded in place inside its square tile
960->480->240->120->60, then one tensor_reduce produces the per-block sums.
All bulk tiles are fp16 (rel err ~5e-4, far inside the 2e-2 gate); the gated
loudness math is f32 with the 1/FRAME scaling folded into the log constants.

The last 960 samples of each row feed no analysis frame and are never loaded.
"""

import math

import numpy as np

import concourse.bass as bass
from concourse import mybir
from concourse.bass_utils import run_bass_kernel_spmd

B, C, T = 16, 32, 192000
N_CORES = 8
ROWS = B * C
RPC = ROWS // N_CORES

FRAME = 9600
SHIFT = 2880
BLK = 960
NBLK_USED = 199          # block 199 (samples 191040..192000) feeds no frame
NFRM = (T - FRAME) // SHIFT + 1  # 64

EPS = 1e-8
ALPHA = 1e-4
GAMMA_A = -70.0
# zs = 9600 * z domain constants
TA_ZS = float(FRAME * (10.0 ** ((GAMMA_A + 0.691) / 10.0) - EPS))
TR_OFF_ZS = float(-0.9 * FRAME * EPS)
EPS_LN = float(FRAME * EPS)
LN10_INV10 = float(10.0 / math.log(10.0))
C_LUFS = float(-0.691 - 10.0 * math.log10(FRAME))

F32 = mybir.dt.float32
F16 = mybir.dt.float16

# ---------------------------------------------------------------- schedule --
# Chunk = contiguous run of 960-sample blocks (all 128 rows).
# kind 'H': fp16 via Pool cast-DMA | 'F': f32 via SP DMA
# sq 'A': ACT square | 'D': DVE square (H only)
# assist: 0 = DVE folds alone; 2/3 = Pool folds in place to 240/120 first


def _schedule():
    chunks = []

    def add(kind, nblk, sq, assist=0):
        chunks.append(
            {"kind": kind, "nblk": nblk, "sq": sq, "assist": assist, "id": len(chunks)}
        )

    # warm-up slices (alternating DVE/ACT squares) so compute starts early
    for k in range(5):
        add("H", 2, "D" if k % 2 == 0 else "A")
    for k in range(14):         # h1..h14; every 3rd is Pool-squared
        add("H", 5, "P" if k % 3 == 2 else "A")
    # late H chunks at 4800, Pool fold-assists to 120-wide; the last 3 are
    # fully Pool-owned (square+folds back-to-back on Pool's queue)
    for k in range(8):
        add("H", 5, "D", assist=2)
    # F region: remaining 79 blocks: 14 x 4800 + tail 4 x 1920 + 960
    for _ in range(14):
        add("F", 5, "A")
    for _ in range(4):
        add("F", 2, "A")
    add("F", 1, "A")

    off = 0
    for ch in chunks:
        if ch["kind"] == "H":
            ch["blk0"] = off
            off += ch["nblk"]
    for ch in chunks:
        if ch["kind"] == "F":
            ch["blk0"] = off
            off += ch["nblk"]
    assert off == NBLK_USED, off
    return chunks


CHUNKS = _schedule()
H_IDS = [c["id"] for c in CHUNKS if c["kind"] == "H"]
F_IDS = [c["id"] for c in CHUNKS if c["kind"] == "F"]
POOL_DMA_ORDER = H_IDS
SP_DMA_ORDER = F_IDS
ASSIST_IDS = [c["id"] for c in CHUNKS if c["assist"]]

# estimated DMA completion times (us-ish units) for ordering heuristics
_t_pool = 0.0
_t_sp = 0.0
READY = {}
for _ch in CHUNKS:
    if _ch["kind"] == "H":
        _t_pool += 7.402 * _ch["nblk"] / 10.0
        READY[_ch["id"]] = _t_pool + 2.9
    else:
        _t_sp += 14.805 * _ch["nblk"] / 10.0
        READY[_ch["id"]] = _t_sp + 2.8

ACT_SQ_ORDER = sorted(
    (c["id"] for c in CHUNKS if c["sq"] == "A"), key=lambda i: READY[i]
)
# DVE-squared chunks in arrival order
D_SEQ = sorted((c["id"] for c in CHUNKS if c["sq"] == "D"), key=lambda i: READY[i])
D_POS = {i: n for n, i in enumerate(D_SEQ)}


def _dve_order():
    # 'sq' jobs at arrival; assisted tree of D-chunk k goes after the square
    # of D-chunk k+2 so every Pool<->DVE wait points backwards in both queues
    jobs = []
    for ch in CHUNKS:
        i = ch["id"]
        if ch["sq"] == "D":
            jobs.append((READY[i], 0, ("sq", i)))
            t = READY[i] + (6.0 if ch["assist"] else 0.5)
            if ch["assist"]:
                k = D_POS[i]
                if k + 2 < len(D_SEQ):
                    t = max(t, READY[D_SEQ[k + 2]] + 0.2)
                else:
                    t = max(t, READY[D_SEQ[-1]] + 0.2 + 0.01 * k)
            jobs.append((t, 1, ("tree", i)))
        elif ch["sq"] == "P":
            jobs.append((READY[i] + (10.0 if ch["assist"] else 8.5), 1, ("tree", i)))
        else:
            jobs.append((READY[i] + 8.3, 1, ("tree", i)))
    jobs.sort()
    return [j for _, _, j in jobs]


DVE_ORDER = _dve_order()
DVE_SQ_SEQ = [i for j, i in DVE_ORDER if j == "sq"]


def _pool_prog():
    # assist for D-chunk at D_SEQ position k rides right after the DMA of
    # D-chunk k+2 (whose square will wait for this assist's chunk tree);
    # Pool squares ('P') ride one DMA after their own
    after_dma = {}
    tail = []
    for a in ASSIST_IDS:
        if a not in D_POS:
            continue  # P-chunk assists are emitted with their psq below
        k = D_POS[a]
        if k + 2 < len(D_SEQ):
            after_dma.setdefault(D_SEQ[k + 2], []).append(("assist", a))
        else:
            tail.append(("assist", a))
    for n, i in enumerate(H_IDS):
        if CHUNKS[i]["sq"] == "P":
            jobs = [("psq", i)]
            if CHUNKS[i]["assist"]:
                jobs.append(("assist", i))
            if n + 1 < len(H_IDS):
                after_dma.setdefault(H_IDS[n + 1], []).extend(jobs)
            else:
                tail.extend(jobs)
    prog = []
    for i in H_IDS:
        prog.append(("dma", i))
        prog.extend(after_dma.get(i, []))
    prog.extend(tail)
    return prog


POOL_PROG = _pool_prog()

NXH = 6   # fp16 input slots [128, 4800]; H squares+folds run in place here
NXF = 6   # f32 input slots [128, 4800]
NSQA = 3  # fp16 square slots for F chunks [128, 4800]


def _build_program() -> bass.Bass:
    nc = bass.Bass("TRN2", target_bir_lowering=False, debug=False)
    AF = mybir.ActivationFunctionType
    ALU = mybir.AluOpType
    AX = mybir.AxisListType

    xy = nc.dram_tensor("xy", [128, T], F32, kind="ExternalInput").ap()
    out = nc.dram_tensor("lufs", [128, 1], F32, kind="ExternalOutput").ap()

    xh = [nc.alloc_sbuf_tensor(f"xh{i}", [128, FRAME // 2], F16).ap() for i in range(NXH)]
    xf = [nc.alloc_sbuf_tensor(f"xf{i}", [128, FRAME // 2], F32).ap() for i in range(NXF)]
    sqA = [nc.alloc_sbuf_tensor(f"sqA{i}", [128, FRAME // 2], F16).ap() for i in range(NSQA)]
    bs = nc.alloc_sbuf_tensor("bs", [128, 200], F16).ap()
    zs = nc.alloc_sbuf_tensor("zs", [128, NFRM], F32).ap()
    ga = nc.alloc_sbuf_tensor("ga", [128, NFRM], F32).ap()
    ma = nc.alloc_sbuf_tensor("ma", [128, NFRM], F32).ap()
    gar = nc.alloc_sbuf_tensor("gar", [128, NFRM], F32).ap()
    sc = nc.alloc_sbuf_tensor("sc", [128, 12], F32).ap()
    eps_t = nc.alloc_sbuf_tensor("eps_t", [128, 1], F32).ap()

    numa, dena, rca, zavea = sc[:, 0:1], sc[:, 1:2], sc[:, 2:3], sc[:, 3:4]
    thr, denar, numar, rcar = sc[:, 4:5], sc[:, 5:6], sc[:, 6:7], sc[:, 7:8]
    zavear, lnz, lufs_t = sc[:, 8:9], sc[:, 9:10], sc[:, 10:11]

    # ---- bookkeeping ----------------------------------------------------
    info = {c["id"]: dict(c) for c in CHUNKS}
    for n, i in enumerate(POOL_DMA_ORDER):
        info[i]["dma_ord"] = n
        info[i]["xslot"] = n % NXH
    for n, i in enumerate(SP_DMA_ORDER):
        info[i]["dma_ord"] = n
        info[i]["xslot"] = n % NXF
    for n, i in enumerate(ACT_SQ_ORDER):
        info[i]["sq_sem"] = "qa"
        info[i]["sq_ord"] = n
    for n, i in enumerate(DVE_SQ_SEQ):
        info[i]["sq_sem"] = "qd"
        info[i]["sq_ord"] = n
    for n, i in enumerate([c["id"] for c in CHUNKS if c["sq"] == "P"]):
        info[i]["sq_sem"] = "qp"
        info[i]["sq_ord"] = n
    F_SQA = [i for i in ACT_SQ_ORDER if info[i]["kind"] == "F"]
    for n, i in enumerate(F_SQA):
        info[i]["sq_slot"] = n % NSQA
        info[i]["fsq_ord"] = n
    _pool_assists = [i for k, i in POOL_PROG if k == "assist"]
    assert sorted(_pool_assists) == sorted(ASSIST_IDS)
    for n, i in enumerate(_pool_assists):
        info[i]["as_ord"] = n
    n = 0
    for job, i in DVE_ORDER:
        if job == "tree":
            info[i]["tree_ord"] = n
            n += 1
    n_trees = n

    def sq_tile(ch):
        # H chunks square and fold in place inside their xh slot
        if ch["kind"] == "H":
            return xh[ch["xslot"]]
        return sqA[ch["sq_slot"]]

    def bs_slice(ch):
        return bs[:, ch["blk0"] : ch["blk0"] + ch["nblk"]]

    def bview(ch, w):
        """[128, nblk, w] view of the chunk's square tile (block stride 960)."""
        t = sq_tile(ch)
        base = t[:, 0:1]
        return type(base)(
            tensor=base.tensor,
            offset=base.offset,
            ap=[list(base.ap[0]), [BLK, ch["nblk"]], [1, w]],
        )

    with (
        nc.Block() as block,
        nc.semaphore("s_pl") as s_pl,
        nc.semaphore("s_sp") as s_sp,
        nc.semaphore("s_qa") as s_qa,
        nc.semaphore("s_qd") as s_qd,
        nc.semaphore("s_tp") as s_tp,
        nc.semaphore("s_qp") as s_qp,
        nc.semaphore("s_tv") as s_tv,
        nc.semaphore("s_ln") as s_ln,
        nc.semaphore("s_out") as s_out,
        nc.allow_low_precision("fp16 block sums; rel err ~5e-4 vs 2e-2 gate"),
    ):
        sems = {"qa": s_qa, "qd": s_qd, "qp": s_qp}

        # ------------------------------------------------------------ Pool --
        @block.gpsimd
        def _(g):
            ndma = 0
            for kind_, i in POOL_PROG:
                ch = info[i]
                nb = ch["nblk"]
                if kind_ == "dma":
                    if ndma >= NXH:
                        prev = info[POOL_DMA_ORDER[ndma - NXH]]
                        g.wait_ge(s_tv, prev["tree_ord"] + 1)
                    b0 = ch["blk0"]
                    g.dma_start(
                        out=xh[ch["xslot"]][:, 0 : nb * BLK],
                        in_=xy[:, b0 * BLK : (b0 + nb) * BLK],
                    ).then_inc(s_pl, 16)
                    ndma += 1
                    continue
                if kind_ == "psq":
                    g.wait_ge(s_pl, 16 * (ch["dma_ord"] + 1))
                    t = xh[ch["xslot"]][:, 0 : nb * BLK]
                    g.tensor_tensor(t, t, t, op=ALU.mult)
                    g.drain().then_inc(s_qp, 1)
                    continue
                # in-place fold assist on the chunk's square tile
                g.wait_ge(sems[ch["sq_sem"]], ch["sq_ord"] + 1)
                g.tensor_tensor(
                    bview(ch, 480), bview(ch, 480),
                    bview(ch, 960)[:, :, 480:960], op=ALU.add,
                )
                if ch["assist"] >= 2:
                    g.drain()
                    g.tensor_tensor(
                        bview(ch, 240), bview(ch, 240),
                        bview(ch, 480)[:, :, 240:480], op=ALU.add,
                    )
                if ch["assist"] >= 3:
                    g.drain()
                    g.tensor_tensor(
                        bview(ch, 120), bview(ch, 120),
                        bview(ch, 240)[:, :, 120:240], op=ALU.add,
                    )
                g.drain().then_inc(s_tp, 1)

        # -------------------------------------------------------------- SP --
        @block.sync
        def _(sync):
            for n, i in enumerate(SP_DMA_ORDER):
                ch = info[i]
                if n >= NXF:
                    prev = info[SP_DMA_ORDER[n - NXF]]
                    sync.wait_ge(sems[prev["sq_sem"]], prev["sq_ord"] + 1)
                b0, nb = ch["blk0"], ch["nblk"]
                sync.dma_start(
                    out=xf[ch["xslot"]][:, 0 : nb * BLK],
                    in_=xy[:, b0 * BLK : (b0 + nb) * BLK],
                ).then_inc(s_sp, 16)
            sync.wait_ge(s_ln, 2)
            sync.dma_start(out=out, in_=lufs_t).then_inc(s_out, 16)
            sync.wait_ge(s_out, 16)

        # ------------------------------------------------------------- ACT --
        @block.scalar
        def _(scalar):
            for n, i in enumerate(ACT_SQ_ORDER):
                ch = info[i]
                nb = ch["nblk"]
                if ch["kind"] == "H":
                    scalar.wait_ge(s_pl, 16 * (ch["dma_ord"] + 1))
                    src = xh[ch["xslot"]]
                else:
                    scalar.wait_ge(s_sp, 16 * (ch["dma_ord"] + 1))
                    src = xf[ch["xslot"]]
                if ch["kind"] == "F" and ch["fsq_ord"] >= NSQA:
                    prev = info[F_SQA[ch["fsq_ord"] - NSQA]]
                    scalar.wait_ge(s_tv, prev["tree_ord"] + 1)
                scalar.activation(
                    sq_tile(ch)[:, 0 : nb * BLK], src[:, 0 : nb * BLK], AF.Square
                )
                scalar.drain().then_inc(s_qa, 1)
            # warm the Ln table while idle so the final Ln is cheap
            scalar.activation(lnz, eps_t, AF.Ln, bias=eps_t)
            scalar.drain()
            scalar.wait_ge(s_tv, n_trees + 1)
            scalar.activation(lnz, zavear, AF.Ln, bias=eps_t)
            scalar.drain().then_inc(s_ln, 1)

        # ------------------------------------------------------------- DVE --
        @block.vector
        def _(vector):
            ALU_ = ALU
            vector.memset(eps_t, EPS_LN)

            def tree_waits(ch):
                if ch["assist"]:
                    vector.wait_ge(s_tp, ch["as_ord"] + 1)
                    return {1: 480, 2: 240, 3: 120}[ch["assist"]]
                if ch["sq_sem"] == "qa":
                    vector.wait_ge(s_qa, ch["sq_ord"] + 1)
                elif ch["sq_sem"] == "qp":
                    vector.wait_ge(s_qp, ch["sq_ord"] + 1)
                return 960

            def emit_trees(group):
                # interleave the chunks' in-place fold chains so one drain
                # covers a whole stage across the group (same-engine RAW is
                # per-chunk; different chunks touch disjoint tiles)
                ws = [tree_waits(c) for c in group]
                while any(w > 60 for w in ws):
                    for k, c in enumerate(group):
                        if ws[k] > 60:
                            h = ws[k] // 2
                            vector.tensor_tensor(
                                bview(c, h), bview(c, h),
                                bview(c, ws[k])[:, :, h : ws[k]], op=ALU_.add,
                            )
                            ws[k] = h
                    vector.drain()
                for c in group:
                    vector.tensor_reduce(
                        bs_slice(c), bview(c, 60), axis=AX.X, op=ALU_.add
                    )
                vector.drain().then_inc(s_tv, len(group))

            def pairable(a, b):
                # only pair late trees whose data is already banked; early
                # pairing couples waits and stalls upstream producers
                return (
                    READY[a["id"]] > 120.0
                    and READY[b["id"]] > 120.0
                    and abs(READY[a["id"]] - READY[b["id"]]) < 8.0
                )

            pending = None
            for job, i in DVE_ORDER:
                ch = info[i]
                nb = ch["nblk"]
                if job == "sq":
                    if pending is not None:
                        emit_trees([pending])
                        pending = None
                    vector.wait_ge(s_pl, 16 * (ch["dma_ord"] + 1))
                    src = xh[ch["xslot"]][:, 0 : nb * BLK]
                    vector.tensor_tensor(
                        sq_tile(ch)[:, 0 : nb * BLK], src, src, op=ALU_.mult
                    )
                    vector.drain().then_inc(s_qd, 1)
                    continue
                if pending is not None:
                    if pairable(pending, ch):
                        emit_trees([pending, ch])
                        pending = None
                    else:
                        emit_trees([pending])
                        pending = ch
                else:
                    pending = ch
            if pending is not None:
                emit_trees([pending])

            # -------------------------------------------------- epilogue ----
            base = bs[:, 0:1]
            frames_view = type(base)(
                tensor=base.tensor,
                offset=base.offset,
                ap=[list(base.ap[0]), [3, NFRM], [1, FRAME // BLK]],
            )
            vector.tensor_reduce(zs, frames_view, axis=AX.X, op=ALU_.add)
            vector.drain()
            vector.scalar_tensor_tensor(
                out=ga, in0=zs, scalar=TA_ZS, in1=zs, op0=ALU_.is_gt, op1=ALU_.mult
            )
            vector.tensor_scalar(ma, zs, TA_ZS, None, op0=ALU_.is_gt)
            vector.drain()
            vector.reduce_sum(numa, ga, axis=AX.X)
            vector.reduce_sum(dena, ma, axis=AX.X)
            vector.drain()
            vector.tensor_scalar_add(dena, dena, EPS)
            vector.drain()
            vector.reciprocal(rca, dena)
            vector.drain()
            vector.tensor_tensor(zavea, numa, rca, op=ALU_.mult)
            vector.drain()
            vector.tensor_scalar(thr, zavea, 0.1, TR_OFF_ZS, op0=ALU_.mult, op1=ALU_.add)
            vector.drain()
            vector.scalar_tensor_tensor(
                out=gar, in0=zs, scalar=thr, in1=ma, op0=ALU_.is_gt, op1=ALU_.mult
            )
            vector.scalar_tensor_tensor(
                out=ga, in0=zs, scalar=thr, in1=ga, op0=ALU_.is_gt, op1=ALU_.mult
            )
            vector.drain()
            vector.reduce_sum(denar, gar, axis=AX.X)
            vector.reduce_sum(numar, ga, axis=AX.X)
            vector.drain()
            vector.tensor_scalar_add(denar, denar, EPS)
            vector.drain()
            vector.reciprocal(rcar, denar)
            vector.drain()
            vector.tensor_tensor(zavear, numar, rcar, op=ALU_.mult)
            vector.drain().then_inc(s_tv, 1)
            vector.wait_ge(s_ln, 1)
            vector.tensor_scalar(
                lufs_t, lnz, LN10_INV10, C_LUFS, op0=ALU_.mult, op1=ALU_.add
            )
            vector.drain().then_inc(s_ln, 1)

    return nc


def make_in_maps(x_env: np.ndarray, y_env: np.ndarray) -> list[dict[str, np.ndarray]]:
    x = np.asarray(x_env, dtype=np.float32).reshape(ROWS, T)
    y = np.asarray(y_env, dtype=np.float32).reshape(ROWS, T)
    in_maps = []
    for i in range(N_CORES):
        shard = np.concatenate(
            [x[i * RPC : (i + 1) * RPC], y[i * RPC : (i + 1) * RPC]], axis=0
        )
        in_maps.append({"xy": np.ascontiguousarray(shard)})
    return in_maps


def finish(per_core_lufs: list[np.ndarray]) -> np.ndarray:
    total = 0.0
    for lf in per_core_lufs:
        lf = np.asarray(lf).reshape(128).astype(np.float64)
        total += np.maximum(lf[RPC:] - lf[:RPC], 0.0).sum()
    return np.array(ALPHA * total, dtype=np.float32)


def kernel(x_env: np.ndarray, y_env: np.ndarray) -> np.ndarray:
    nc = _build_program()
    in_maps = make_in_maps(x_env, y_env)
    res = run_bass_kernel_spmd(nc, in_maps, core_ids=list(range(N_CORES)))
    return finish([res.results[i]["lufs"] for i in range(N_CORES)])


# revision 14
# speedup vs baseline: 1.0226x; 1.0073x over previous
"""DHASPI level-loss kernel v2 for 8 Trainium2 NeuronCores.

Data-parallel over the fused B*C row axis (64 x-rows + 64 y-rows per core in
the 128 SBUF partitions). The work is spread across all four engine queues:

- Pool (gpsimd SWDGE): casting DMAs f32 HBM -> fp16 SBUF (half the modeled
  DMA cost) for "H" chunks, plus in-place fold-assists on late chunks.
- SP (sync HWDGE): f32 DMAs for "F" chunks, running concurrently with Pool's.
- ACT: squares (f32 or fp16 in -> fp16 out) + the final Ln.
- DVE: fp16 squares of some H chunks (2x perf mode), block-sum fold trees
  (in-place halving adds at 2x + a final 60-wide tensor_reduce), epilogue.

Block sums: each 960-sample block is folded in place inside its square tile
960->480->240->120->60, then one tensor_reduce produces the per-block sums.
All bulk tiles are fp16 (rel err ~5e-4, far inside the 2e-2 gate); the gated
loudness math is f32 with the 1/FRAME scaling folded into the log constants.

The last 960 samples of each row feed no analysis frame and are never loaded.
"""

import math

import numpy as np

import concourse.bass as bass
from concourse import mybir
from concourse.bass_utils import run_bass_kernel_spmd

B, C, T = 16, 32, 192000
N_CORES = 8
ROWS = B * C
RPC = ROWS // N_CORES

FRAME = 9600
SHIFT = 2880
BLK = 960
NBLK_USED = 199          # block 199 (samples 191040..192000) feeds no frame
NFRM = (T - FRAME) // SHIFT + 1  # 64

EPS = 1e-8
ALPHA = 1e-4
GAMMA_A = -70.0
# zs = 9600 * z domain constants
TA_ZS = float(FRAME * (10.0 ** ((GAMMA_A + 0.691) / 10.0) - EPS))
TR_OFF_ZS = float(-0.9 * FRAME * EPS)
EPS_LN = float(FRAME * EPS)
LN10_INV10 = float(10.0 / math.log(10.0))
C_LUFS = float(-0.691 - 10.0 * math.log10(FRAME))

F32 = mybir.dt.float32
F16 = mybir.dt.float16

# ---------------------------------------------------------------- schedule --
# Chunk = contiguous run of 960-sample blocks (all 128 rows).
# kind 'H': fp16 via Pool cast-DMA | 'F': f32 via SP DMA
# sq 'A': ACT square | 'D': DVE square (H only)
# assist: 0 = DVE folds alone; 2/3 = Pool folds in place to 240/120 first


def _schedule():
    chunks = []

    def add(kind, nblk, sq, assist=0):
        chunks.append(
            {"kind": kind, "nblk": nblk, "sq": sq, "assist": assist, "id": len(chunks)}
        )

    # warm-up slices (alternating DVE/ACT squares) so compute starts early
    for k in range(5):
        add("H", 2, "D" if k % 2 == 0 else "A")
    for k in range(14):         # h1..h14; every 3rd is Pool-squared
        add("H", 5, "P" if k % 3 == 2 else "A")
    # late H chunks at 4800, Pool fold-assists to 120-wide; the last 3 are
    # fully Pool-owned (square+folds back-to-back on Pool's queue)
    for k in range(8):
        add("H", 5, "D", assist=4 if k == 7 else (3 if k >= 6 else 2))
    # F region: remaining 79 blocks: 14 x 4800 + tail 4 x 1920 + 960
    for _ in range(14):
        add("F", 5, "A")
    for _ in range(4):
        add("F", 2, "A")
    add("F", 1, "A")

    off = 0
    for ch in chunks:
        if ch["kind"] == "H":
            ch["blk0"] = off
            off += ch["nblk"]
    for ch in chunks:
        if ch["kind"] == "F":
            ch["blk0"] = off
            off += ch["nblk"]
    assert off == NBLK_USED, off
    return chunks


CHUNKS = _schedule()
H_IDS = [c["id"] for c in CHUNKS if c["kind"] == "H"]
F_IDS = [c["id"] for c in CHUNKS if c["kind"] == "F"]
POOL_DMA_ORDER = H_IDS
SP_DMA_ORDER = F_IDS
ASSIST_IDS = [c["id"] for c in CHUNKS if c["assist"]]

# estimated DMA completion times (us-ish units) for ordering heuristics
_t_pool = 0.0
_t_sp = 0.0
READY = {}
for _ch in CHUNKS:
    if _ch["kind"] == "H":
        _t_pool += 7.402 * _ch["nblk"] / 10.0
        READY[_ch["id"]] = _t_pool + 2.9
    else:
        _t_sp += 14.805 * _ch["nblk"] / 10.0
        READY[_ch["id"]] = _t_sp + 2.8

ACT_SQ_ORDER = sorted(
    (c["id"] for c in CHUNKS if c["sq"] == "A"), key=lambda i: READY[i]
)
# DVE-squared chunks in arrival order
D_SEQ = sorted((c["id"] for c in CHUNKS if c["sq"] == "D"), key=lambda i: READY[i])
D_POS = {i: n for n, i in enumerate(D_SEQ)}


def _dve_order():
    # 'sq' jobs at arrival; assisted tree of D-chunk k goes after the square
    # of D-chunk k+2 so every Pool<->DVE wait points backwards in both queues
    jobs = []
    for ch in CHUNKS:
        i = ch["id"]
        if ch["sq"] == "D":
            jobs.append((READY[i], 0, ("sq", i)))
            t = READY[i] + (6.0 if ch["assist"] else 0.5)
            if ch["assist"]:
                k = D_POS[i]
                if k + 2 < len(D_SEQ):
                    t = max(t, READY[D_SEQ[k + 2]] + 0.2)
                else:
                    t = max(t, READY[D_SEQ[-1]] + 0.2 + 0.01 * k)
            jobs.append((t, 1, ("tree", i)))
        elif ch["sq"] == "P":
            jobs.append((READY[i] + (10.0 if ch["assist"] else 8.5), 1, ("tree", i)))
        else:
            jobs.append((READY[i] + 8.3, 1, ("tree", i)))
    jobs.sort()
    return [j for _, _, j in jobs]


DVE_ORDER = _dve_order()
DVE_SQ_SEQ = [i for j, i in DVE_ORDER if j == "sq"]


def _pool_prog():
    # assist for D-chunk at D_SEQ position k rides right after the DMA of
    # D-chunk k+2 (whose square will wait for this assist's chunk tree);
    # Pool squares ('P') ride one DMA after their own
    after_dma = {}
    tail = []
    for a in ASSIST_IDS:
        if a not in D_POS:
            continue  # P-chunk assists are emitted with their psq below
        k = D_POS[a]
        if k + 2 < len(D_SEQ):
            after_dma.setdefault(D_SEQ[k + 2], []).append(("assist", a))
        else:
            tail.append(("assist", a))
    for n, i in enumerate(H_IDS):
        if CHUNKS[i]["sq"] == "P":
            jobs = [("psq", i)]
            if CHUNKS[i]["assist"]:
                jobs.append(("assist", i))
            if n + 1 < len(H_IDS):
                after_dma.setdefault(H_IDS[n + 1], []).extend(jobs)
            else:
                tail.extend(jobs)
    prog = []
    for i in H_IDS:
        prog.append(("dma", i))
        prog.extend(after_dma.get(i, []))
    prog.extend(tail)
    return prog


POOL_PROG = _pool_prog()

NXH = 6   # fp16 input slots [128, 4800]; H squares+folds run in place here
NXF = 6   # f32 input slots [128, 4800]
NSQA = 3  # fp16 square slots for F chunks [128, 4800]


def _build_program() -> bass.Bass:
    nc = bass.Bass("TRN2", target_bir_lowering=False, debug=False)
    AF = mybir.ActivationFunctionType
    ALU = mybir.AluOpType
    AX = mybir.AxisListType

    xy = nc.dram_tensor("xy", [128, T], F32, kind="ExternalInput").ap()
    out = nc.dram_tensor("lufs", [128, 1], F32, kind="ExternalOutput").ap()

    xh = [nc.alloc_sbuf_tensor(f"xh{i}", [128, FRAME // 2], F16).ap() for i in range(NXH)]
    xf = [nc.alloc_sbuf_tensor(f"xf{i}", [128, FRAME // 2], F32).ap() for i in range(NXF)]
    sqA = [nc.alloc_sbuf_tensor(f"sqA{i}", [128, FRAME // 2], F16).ap() for i in range(NSQA)]
    bs = nc.alloc_sbuf_tensor("bs", [128, 200], F16).ap()
    zs = nc.alloc_sbuf_tensor("zs", [128, NFRM], F32).ap()
    ga = nc.alloc_sbuf_tensor("ga", [128, NFRM], F32).ap()
    ma = nc.alloc_sbuf_tensor("ma", [128, NFRM], F32).ap()
    gar = nc.alloc_sbuf_tensor("gar", [128, NFRM], F32).ap()
    sc = nc.alloc_sbuf_tensor("sc", [128, 12], F32).ap()
    eps_t = nc.alloc_sbuf_tensor("eps_t", [128, 1], F32).ap()

    numa, dena, rca, zavea = sc[:, 0:1], sc[:, 1:2], sc[:, 2:3], sc[:, 3:4]
    thr, denar, numar, rcar = sc[:, 4:5], sc[:, 5:6], sc[:, 6:7], sc[:, 7:8]
    zavear, lnz, lufs_t = sc[:, 8:9], sc[:, 9:10], sc[:, 10:11]

    # ---- bookkeeping ----------------------------------------------------
    info = {c["id"]: dict(c) for c in CHUNKS}
    for n, i in enumerate(POOL_DMA_ORDER):
        info[i]["dma_ord"] = n
        info[i]["xslot"] = n % NXH
    for n, i in enumerate(SP_DMA_ORDER):
        info[i]["dma_ord"] = n
        info[i]["xslot"] = n % NXF
    for n, i in enumerate(ACT_SQ_ORDER):
        info[i]["sq_sem"] = "qa"
        info[i]["sq_ord"] = n
    for n, i in enumerate(DVE_SQ_SEQ):
        info[i]["sq_sem"] = "qd"
        info[i]["sq_ord"] = n
    for n, i in enumerate([c["id"] for c in CHUNKS if c["sq"] == "P"]):
        info[i]["sq_sem"] = "qp"
        info[i]["sq_ord"] = n
    F_SQA = [i for i in ACT_SQ_ORDER if info[i]["kind"] == "F"]
    for n, i in enumerate(F_SQA):
        info[i]["sq_slot"] = n % NSQA
        info[i]["fsq_ord"] = n
    _pool_assists = [i for k, i in POOL_PROG if k == "assist"]
    assert sorted(_pool_assists) == sorted(ASSIST_IDS)
    for n, i in enumerate(_pool_assists):
        info[i]["as_ord"] = n
    n = 0
    for job, i in DVE_ORDER:
        if job == "tree":
            info[i]["tree_ord"] = n
            n += 1
    n_trees = n

    def sq_tile(ch):
        # H chunks square and fold in place inside their xh slot
        if ch["kind"] == "H":
            return xh[ch["xslot"]]
        return sqA[ch["sq_slot"]]

    def bs_slice(ch):
        return bs[:, ch["blk0"] : ch["blk0"] + ch["nblk"]]

    def bview(ch, w):
        """[128, nblk, w] view of the chunk's square tile (block stride 960)."""
        t = sq_tile(ch)
        base = t[:, 0:1]
        return type(base)(
            tensor=base.tensor,
            offset=base.offset,
            ap=[list(base.ap[0]), [BLK, ch["nblk"]], [1, w]],
        )

    with (
        nc.Block() as block,
        nc.semaphore("s_pl") as s_pl,
        nc.semaphore("s_sp") as s_sp,
        nc.semaphore("s_qa") as s_qa,
        nc.semaphore("s_qd") as s_qd,
        nc.semaphore("s_tp") as s_tp,
        nc.semaphore("s_qp") as s_qp,
        nc.semaphore("s_tv") as s_tv,
        nc.semaphore("s_ln") as s_ln,
        nc.semaphore("s_out") as s_out,
        nc.allow_low_precision("fp16 block sums; rel err ~5e-4 vs 2e-2 gate"),
    ):
        sems = {"qa": s_qa, "qd": s_qd, "qp": s_qp}

        # ------------------------------------------------------------ Pool --
        @block.gpsimd
        def _(g):
            ndma = 0
            for kind_, i in POOL_PROG:
                ch = info[i]
                nb = ch["nblk"]
                if kind_ == "dma":
                    if ndma >= NXH:
                        prev = info[POOL_DMA_ORDER[ndma - NXH]]
                        g.wait_ge(s_tv, prev["tree_ord"] + 1)
                    b0 = ch["blk0"]
                    g.dma_start(
                        out=xh[ch["xslot"]][:, 0 : nb * BLK],
                        in_=xy[:, b0 * BLK : (b0 + nb) * BLK],
                    ).then_inc(s_pl, 16)
                    ndma += 1
                    continue
                if kind_ == "psq":
                    g.wait_ge(s_pl, 16 * (ch["dma_ord"] + 1))
                    t = xh[ch["xslot"]][:, 0 : nb * BLK]
                    g.tensor_tensor(t, t, t, op=ALU.mult)
                    g.drain().then_inc(s_qp, 1)
                    continue
                # in-place fold assist on the chunk's square tile
                g.wait_ge(sems[ch["sq_sem"]], ch["sq_ord"] + 1)
                g.tensor_tensor(
                    bview(ch, 480), bview(ch, 480),
                    bview(ch, 960)[:, :, 480:960], op=ALU.add,
                )
                if ch["assist"] >= 2:
                    g.drain()
                    g.tensor_tensor(
                        bview(ch, 240), bview(ch, 240),
                        bview(ch, 480)[:, :, 240:480], op=ALU.add,
                    )
                if ch["assist"] >= 3:
                    g.drain()
                    g.tensor_tensor(
                        bview(ch, 120), bview(ch, 120),
                        bview(ch, 240)[:, :, 120:240], op=ALU.add,
                    )
                if ch["assist"] >= 4:
                    g.drain()
                    g.tensor_tensor(
                        bview(ch, 60), bview(ch, 60),
                        bview(ch, 120)[:, :, 60:120], op=ALU.add,
                    )
                g.drain().then_inc(s_tp, 1)

        # -------------------------------------------------------------- SP --
        @block.sync
        def _(sync):
            for n, i in enumerate(SP_DMA_ORDER):
                ch = info[i]
                if n >= NXF:
                    prev = info[SP_DMA_ORDER[n - NXF]]
                    sync.wait_ge(sems[prev["sq_sem"]], prev["sq_ord"] + 1)
                b0, nb = ch["blk0"], ch["nblk"]
                sync.dma_start(
                    out=xf[ch["xslot"]][:, 0 : nb * BLK],
                    in_=xy[:, b0 * BLK : (b0 + nb) * BLK],
                ).then_inc(s_sp, 16)
            sync.wait_ge(s_ln, 2)
            sync.dma_start(out=out, in_=lufs_t).then_inc(s_out, 16)
            sync.wait_ge(s_out, 16)

        # ------------------------------------------------------------- ACT --
        @block.scalar
        def _(scalar):
            for n, i in enumerate(ACT_SQ_ORDER):
                ch = info[i]
                nb = ch["nblk"]
                if ch["kind"] == "H":
                    scalar.wait_ge(s_pl, 16 * (ch["dma_ord"] + 1))
                    src = xh[ch["xslot"]]
                else:
                    scalar.wait_ge(s_sp, 16 * (ch["dma_ord"] + 1))
                    src = xf[ch["xslot"]]
                if ch["kind"] == "F" and ch["fsq_ord"] >= NSQA:
                    prev = info[F_SQA[ch["fsq_ord"] - NSQA]]
                    scalar.wait_ge(s_tv, prev["tree_ord"] + 1)
                scalar.activation(
                    sq_tile(ch)[:, 0 : nb * BLK], src[:, 0 : nb * BLK], AF.Square
                )
                scalar.drain().then_inc(s_qa, 1)
            # warm the Ln table while idle so the final Ln is cheap
            scalar.activation(lnz, eps_t, AF.Ln, bias=eps_t)
            scalar.drain()
            scalar.wait_ge(s_tv, n_trees + 1)
            scalar.activation(lnz, zavear, AF.Ln, bias=eps_t)
            scalar.drain().then_inc(s_ln, 1)

        # ------------------------------------------------------------- DVE --
        @block.vector
        def _(vector):
            ALU_ = ALU
            vector.memset(eps_t, EPS_LN)

            def tree_waits(ch):
                if ch["assist"]:
                    vector.wait_ge(s_tp, ch["as_ord"] + 1)
                    return {1: 480, 2: 240, 3: 120, 4: 60}[ch["assist"]]
                if ch["sq_sem"] == "qa":
                    vector.wait_ge(s_qa, ch["sq_ord"] + 1)
                elif ch["sq_sem"] == "qp":
                    vector.wait_ge(s_qp, ch["sq_ord"] + 1)
                return 960

            def emit_trees(group):
                # interleave the chunks' in-place fold chains so one drain
                # covers a whole stage across the group (same-engine RAW is
                # per-chunk; different chunks touch disjoint tiles)
                ws = [tree_waits(c) for c in group]
                while any(w > 60 for w in ws):
                    for k, c in enumerate(group):
                        if ws[k] > 60:
                            h = ws[k] // 2
                            vector.tensor_tensor(
                                bview(c, h), bview(c, h),
                                bview(c, ws[k])[:, :, h : ws[k]], op=ALU_.add,
                            )
                            ws[k] = h
                    vector.drain()
                for c in group:
                    vector.tensor_reduce(
                        bs_slice(c), bview(c, 60), axis=AX.X, op=ALU_.add
                    )
                vector.drain().then_inc(s_tv, len(group))

            def pairable(a, b):
                # only pair late trees whose data is already banked; early
                # pairing couples waits and stalls upstream producers
                return (
                    READY[a["id"]] > 120.0
                    and READY[b["id"]] > 120.0
                    and abs(READY[a["id"]] - READY[b["id"]]) < 8.0
                )

            pending = None
            for job, i in DVE_ORDER:
                ch = info[i]
                nb = ch["nblk"]
                if job == "sq":
                    if pending is not None:
                        emit_trees([pending])
                        pending = None
                    vector.wait_ge(s_pl, 16 * (ch["dma_ord"] + 1))
                    src = xh[ch["xslot"]][:, 0 : nb * BLK]
                    vector.tensor_tensor(
                        sq_tile(ch)[:, 0 : nb * BLK], src, src, op=ALU_.mult
                    )
                    vector.drain().then_inc(s_qd, 1)
                    continue
                if pending is not None:
                    if pairable(pending, ch):
                        emit_trees([pending, ch])
                        pending = None
                    else:
                        emit_trees([pending])
                        pending = ch
                else:
                    pending = ch
            if pending is not None:
                emit_trees([pending])

            # -------------------------------------------------- epilogue ----
            base = bs[:, 0:1]
            frames_view = type(base)(
                tensor=base.tensor,
                offset=base.offset,
                ap=[list(base.ap[0]), [3, NFRM], [1, FRAME // BLK]],
            )
            vector.tensor_reduce(zs, frames_view, axis=AX.X, op=ALU_.add)
            vector.drain()
            vector.scalar_tensor_tensor(
                out=ga, in0=zs, scalar=TA_ZS, in1=zs, op0=ALU_.is_gt, op1=ALU_.mult
            )
            vector.tensor_scalar(ma, zs, TA_ZS, None, op0=ALU_.is_gt)
            vector.drain()
            vector.reduce_sum(numa, ga, axis=AX.X)
            vector.reduce_sum(dena, ma, axis=AX.X)
            vector.drain()
            vector.tensor_scalar_add(dena, dena, EPS)
            vector.drain()
            vector.reciprocal(rca, dena)
            vector.drain()
            vector.tensor_tensor(zavea, numa, rca, op=ALU_.mult)
            vector.drain()
            vector.tensor_scalar(thr, zavea, 0.1, TR_OFF_ZS, op0=ALU_.mult, op1=ALU_.add)
            vector.drain()
            vector.scalar_tensor_tensor(
                out=gar, in0=zs, scalar=thr, in1=ma, op0=ALU_.is_gt, op1=ALU_.mult
            )
            vector.scalar_tensor_tensor(
                out=ga, in0=zs, scalar=thr, in1=ga, op0=ALU_.is_gt, op1=ALU_.mult
            )
            vector.drain()
            vector.reduce_sum(denar, gar, axis=AX.X)
            vector.reduce_sum(numar, ga, axis=AX.X)
            vector.drain()
            vector.tensor_scalar_add(denar, denar, EPS)
            vector.drain()
            vector.reciprocal(rcar, denar)
            vector.drain()
            vector.tensor_tensor(zavear, numar, rcar, op=ALU_.mult)
            vector.drain().then_inc(s_tv, 1)
            vector.wait_ge(s_ln, 1)
            vector.tensor_scalar(
                lufs_t, lnz, LN10_INV10, C_LUFS, op0=ALU_.mult, op1=ALU_.add
            )
            vector.drain().then_inc(s_ln, 1)

    return nc


def make_in_maps(x_env: np.ndarray, y_env: np.ndarray) -> list[dict[str, np.ndarray]]:
    x = np.asarray(x_env, dtype=np.float32).reshape(ROWS, T)
    y = np.asarray(y_env, dtype=np.float32).reshape(ROWS, T)
    in_maps = []
    for i in range(N_CORES):
        shard = np.concatenate(
            [x[i * RPC : (i + 1) * RPC], y[i * RPC : (i + 1) * RPC]], axis=0
        )
        in_maps.append({"xy": np.ascontiguousarray(shard)})
    return in_maps


def finish(per_core_lufs: list[np.ndarray]) -> np.ndarray:
    total = 0.0
    for lf in per_core_lufs:
        lf = np.asarray(lf).reshape(128).astype(np.float64)
        total += np.maximum(lf[RPC:] - lf[:RPC], 0.0).sum()
    return np.array(ALPHA * total, dtype=np.float32)


def kernel(x_env: np.ndarray, y_env: np.ndarray) -> np.ndarray:
    nc = _build_program()
    in_maps = make_in_maps(x_env, y_env)
    res = run_bass_kernel_spmd(nc, in_maps, core_ids=list(range(N_CORES)))
    return finish([res.results[i]["lufs"] for i in range(N_CORES)])


# revision 15
# speedup vs baseline: 1.0239x; 1.0013x over previous
"""DHASPI level-loss kernel v2 for 8 Trainium2 NeuronCores.

Data-parallel over the fused B*C row axis (64 x-rows + 64 y-rows per core in
the 128 SBUF partitions). The work is spread across all four engine queues:

- Pool (gpsimd SWDGE): casting DMAs f32 HBM -> fp16 SBUF (half the modeled
  DMA cost) for "H" chunks, plus in-place fold-assists on late chunks.
- SP (sync HWDGE): f32 DMAs for "F" chunks, running concurrently with Pool's.
- ACT: squares (f32 or fp16 in -> fp16 out) + the final Ln.
- DVE: fp16 squares of some H chunks (2x perf mode), block-sum fold trees
  (in-place halving adds at 2x + a final 60-wide tensor_reduce), epilogue.

Block sums: each 960-sample block is folded in place inside its square tile
960->480->240->120->60, then one tensor_reduce produces the per-block sums.
All bulk tiles are fp16 (rel err ~5e-4, far inside the 2e-2 gate); the gated
loudness math is f32 with the 1/FRAME scaling folded into the log constants.

The last 960 samples of each row feed no analysis frame and are never loaded.
"""

import math

import numpy as np

import concourse.bass as bass
from concourse import mybir
from concourse.bass_utils import run_bass_kernel_spmd

B, C, T = 16, 32, 192000
N_CORES = 8
ROWS = B * C
RPC = ROWS // N_CORES

FRAME = 9600
SHIFT = 2880
BLK = 960
NBLK_USED = 199          # block 199 (samples 191040..192000) feeds no frame
NFRM = (T - FRAME) // SHIFT + 1  # 64

EPS = 1e-8
ALPHA = 1e-4
GAMMA_A = -70.0
# zs = 9600 * z domain constants
TA_ZS = float(FRAME * (10.0 ** ((GAMMA_A + 0.691) / 10.0) - EPS))
TR_OFF_ZS = float(-0.9 * FRAME * EPS)
EPS_LN = float(FRAME * EPS)
LN10_INV10 = float(10.0 / math.log(10.0))
C_LUFS = float(-0.691 - 10.0 * math.log10(FRAME))

F32 = mybir.dt.float32
F16 = mybir.dt.float16

# ---------------------------------------------------------------- schedule --
# Chunk = contiguous run of 960-sample blocks (all 128 rows).
# kind 'H': fp16 via Pool cast-DMA | 'F': f32 via SP DMA
# sq 'A': ACT square | 'D': DVE square (H only)
# assist: 0 = DVE folds alone; 2/3 = Pool folds in place to 240/120 first


def _schedule():
    chunks = []

    def add(kind, nblk, sq, assist=0):
        chunks.append(
            {"kind": kind, "nblk": nblk, "sq": sq, "assist": assist, "id": len(chunks)}
        )

    # warm-up slices (alternating DVE/ACT squares) so compute starts early
    for k in range(5):
        add("H", 2, "D" if k % 2 == 0 else "A")
    for k in range(14):         # h1..h14; every 3rd is Pool-squared
        add("H", 5, "P" if k % 3 == 2 else "A")
    # late H chunks at 4800, Pool fold-assists to 120-wide; the last 3 are
    # fully Pool-owned (square+folds back-to-back on Pool's queue)
    for k in range(8):
        add("H", 5, "D", assist=4 if k == 7 else (3 if k >= 6 else 2))
    # F region: remaining 79 blocks: 14 x 4800 + tail 4 x 1920 + 960
    for _ in range(14):
        add("F", 5, "A")
    for _ in range(4):
        add("F", 2, "A")
    add("F", 1, "A")

    off = 0
    for ch in chunks:
        if ch["kind"] == "H":
            ch["blk0"] = off
            off += ch["nblk"]
    for ch in chunks:
        if ch["kind"] == "F":
            ch["blk0"] = off
            off += ch["nblk"]
    assert off == NBLK_USED, off
    return chunks


CHUNKS = _schedule()
H_IDS = [c["id"] for c in CHUNKS if c["kind"] == "H"]
F_IDS = [c["id"] for c in CHUNKS if c["kind"] == "F"]
POOL_DMA_ORDER = H_IDS
SP_DMA_ORDER = F_IDS
ASSIST_IDS = [c["id"] for c in CHUNKS if c["assist"]]

# estimated DMA completion times (us-ish units) for ordering heuristics
_t_pool = 0.0
_t_sp = 0.0
READY = {}
for _ch in CHUNKS:
    if _ch["kind"] == "H":
        _t_pool += 7.402 * _ch["nblk"] / 10.0
        READY[_ch["id"]] = _t_pool + 2.9
    else:
        _t_sp += 14.805 * _ch["nblk"] / 10.0
        READY[_ch["id"]] = _t_sp + 2.8

ACT_SQ_ORDER = sorted(
    (c["id"] for c in CHUNKS if c["sq"] == "A"), key=lambda i: READY[i]
)
# DVE-squared chunks in arrival order
D_SEQ = sorted((c["id"] for c in CHUNKS if c["sq"] == "D"), key=lambda i: READY[i])
D_POS = {i: n for n, i in enumerate(D_SEQ)}


def _dve_order():
    # 'sq' jobs at arrival; assisted tree of D-chunk k goes after the square
    # of D-chunk k+2 so every Pool<->DVE wait points backwards in both queues
    jobs = []
    for ch in CHUNKS:
        i = ch["id"]
        if ch["sq"] == "D":
            jobs.append((READY[i], 0, ("sq", i)))
            t = READY[i] + (6.0 if ch["assist"] else 0.5)
            if ch["assist"]:
                k = D_POS[i]
                if k + 2 < len(D_SEQ):
                    t = max(t, READY[D_SEQ[k + 2]] + 0.2)
                else:
                    t = max(t, READY[D_SEQ[-1]] + 0.2 + 0.01 * k)
            jobs.append((t, 1, ("tree", i)))
        elif ch["sq"] == "P":
            jobs.append((READY[i] + (10.0 if ch["assist"] else 8.5), 1, ("tree", i)))
        else:
            jobs.append((READY[i] + 8.3, 1, ("tree", i)))
    jobs.sort()
    return [j for _, _, j in jobs]


DVE_ORDER = _dve_order()
DVE_SQ_SEQ = [i for j, i in DVE_ORDER if j == "sq"]


def _pool_prog():
    # assist for D-chunk at D_SEQ position k rides right after the DMA of
    # D-chunk k+2 (whose square will wait for this assist's chunk tree);
    # Pool squares ('P') ride one DMA after their own
    after_dma = {}
    tail = []
    for a in ASSIST_IDS:
        if a not in D_POS:
            continue  # P-chunk assists are emitted with their psq below
        k = D_POS[a]
        if k + 2 < len(D_SEQ):
            after_dma.setdefault(D_SEQ[k + 2], []).append(("assist", a))
        else:
            tail.append(("assist", a))
    for n, i in enumerate(H_IDS):
        if CHUNKS[i]["sq"] == "P":
            jobs = [("psq", i)]
            if CHUNKS[i]["assist"]:
                jobs.append(("assist", i))
            if n + 1 < len(H_IDS):
                after_dma.setdefault(H_IDS[n + 1], []).extend(jobs)
            else:
                tail.extend(jobs)
    prog = []
    for i in H_IDS:
        prog.append(("dma", i))
        prog.extend(after_dma.get(i, []))
    prog.extend(tail)
    return prog


POOL_PROG = _pool_prog()

NXH = 6   # fp16 input slots [128, 4800]; H squares+folds run in place here
NXF = 6   # f32 input slots [128, 4800]
NSQA = 3  # fp16 square slots for F chunks [128, 4800]


def _build_program() -> bass.Bass:
    nc = bass.Bass("TRN2", target_bir_lowering=False, debug=False)
    AF = mybir.ActivationFunctionType
    ALU = mybir.AluOpType
    AX = mybir.AxisListType

    xy = nc.dram_tensor("xy", [128, T], F32, kind="ExternalInput").ap()
    out = nc.dram_tensor("lufs", [128, 1], F32, kind="ExternalOutput").ap()

    xh = [nc.alloc_sbuf_tensor(f"xh{i}", [128, FRAME // 2], F16).ap() for i in range(NXH)]
    xf = [nc.alloc_sbuf_tensor(f"xf{i}", [128, FRAME // 2], F32).ap() for i in range(NXF)]
    sqA = [nc.alloc_sbuf_tensor(f"sqA{i}", [128, FRAME // 2], F16).ap() for i in range(NSQA)]
    bs = nc.alloc_sbuf_tensor("bs", [128, 200], F16).ap()
    zs = nc.alloc_sbuf_tensor("zs", [128, NFRM], F32).ap()
    ga = nc.alloc_sbuf_tensor("ga", [128, NFRM], F32).ap()
    ma = nc.alloc_sbuf_tensor("ma", [128, NFRM], F32).ap()
    gar = nc.alloc_sbuf_tensor("gar", [128, NFRM], F32).ap()
    sc = nc.alloc_sbuf_tensor("sc", [128, 12], F32).ap()

    numa, dena, rca, zavea = sc[:, 0:1], sc[:, 1:2], sc[:, 2:3], sc[:, 3:4]
    thr, denar, numar, rcar = sc[:, 4:5], sc[:, 5:6], sc[:, 6:7], sc[:, 7:8]
    zavear, lnz, lufs_t = sc[:, 8:9], sc[:, 9:10], sc[:, 10:11]

    # ---- bookkeeping ----------------------------------------------------
    info = {c["id"]: dict(c) for c in CHUNKS}
    for n, i in enumerate(POOL_DMA_ORDER):
        info[i]["dma_ord"] = n
        info[i]["xslot"] = n % NXH
    for n, i in enumerate(SP_DMA_ORDER):
        info[i]["dma_ord"] = n
        info[i]["xslot"] = n % NXF
    for n, i in enumerate(ACT_SQ_ORDER):
        info[i]["sq_sem"] = "qa"
        info[i]["sq_ord"] = n
    for n, i in enumerate(DVE_SQ_SEQ):
        info[i]["sq_sem"] = "qd"
        info[i]["sq_ord"] = n
    for n, i in enumerate([c["id"] for c in CHUNKS if c["sq"] == "P"]):
        info[i]["sq_sem"] = "qp"
        info[i]["sq_ord"] = n
    F_SQA = [i for i in ACT_SQ_ORDER if info[i]["kind"] == "F"]
    for n, i in enumerate(F_SQA):
        info[i]["sq_slot"] = n % NSQA
        info[i]["fsq_ord"] = n
    _pool_assists = [i for k, i in POOL_PROG if k == "assist"]
    assert sorted(_pool_assists) == sorted(ASSIST_IDS)
    for n, i in enumerate(_pool_assists):
        info[i]["as_ord"] = n
    n = 0
    for job, i in DVE_ORDER:
        if job == "tree":
            info[i]["tree_ord"] = n
            n += 1
    n_trees = n

    def sq_tile(ch):
        # H chunks square and fold in place inside their xh slot
        if ch["kind"] == "H":
            return xh[ch["xslot"]]
        return sqA[ch["sq_slot"]]

    def bs_slice(ch):
        return bs[:, ch["blk0"] : ch["blk0"] + ch["nblk"]]

    def bview(ch, w):
        """[128, nblk, w] view of the chunk's square tile (block stride 960)."""
        t = sq_tile(ch)
        base = t[:, 0:1]
        return type(base)(
            tensor=base.tensor,
            offset=base.offset,
            ap=[list(base.ap[0]), [BLK, ch["nblk"]], [1, w]],
        )

    with (
        nc.Block() as block,
        nc.semaphore("s_pl") as s_pl,
        nc.semaphore("s_sp") as s_sp,
        nc.semaphore("s_qa") as s_qa,
        nc.semaphore("s_qd") as s_qd,
        nc.semaphore("s_tp") as s_tp,
        nc.semaphore("s_qp") as s_qp,
        nc.semaphore("s_tv") as s_tv,
        nc.semaphore("s_ln") as s_ln,
        nc.semaphore("s_out") as s_out,
        nc.allow_low_precision("fp16 block sums; rel err ~5e-4 vs 2e-2 gate"),
    ):
        sems = {"qa": s_qa, "qd": s_qd, "qp": s_qp}

        # ------------------------------------------------------------ Pool --
        @block.gpsimd
        def _(g):
            ndma = 0
            for kind_, i in POOL_PROG:
                ch = info[i]
                nb = ch["nblk"]
                if kind_ == "dma":
                    if ndma >= NXH:
                        prev = info[POOL_DMA_ORDER[ndma - NXH]]
                        g.wait_ge(s_tv, prev["tree_ord"] + 1)
                    b0 = ch["blk0"]
                    g.dma_start(
                        out=xh[ch["xslot"]][:, 0 : nb * BLK],
                        in_=xy[:, b0 * BLK : (b0 + nb) * BLK],
                    ).then_inc(s_pl, 16)
                    ndma += 1
                    continue
                if kind_ == "psq":
                    g.wait_ge(s_pl, 16 * (ch["dma_ord"] + 1))
                    t = xh[ch["xslot"]][:, 0 : nb * BLK]
                    g.tensor_tensor(t, t, t, op=ALU.mult)
                    g.drain().then_inc(s_qp, 1)
                    continue
                # in-place fold assist on the chunk's square tile
                g.wait_ge(sems[ch["sq_sem"]], ch["sq_ord"] + 1)
                g.tensor_tensor(
                    bview(ch, 480), bview(ch, 480),
                    bview(ch, 960)[:, :, 480:960], op=ALU.add,
                )
                if ch["assist"] >= 2:
                    g.drain()
                    g.tensor_tensor(
                        bview(ch, 240), bview(ch, 240),
                        bview(ch, 480)[:, :, 240:480], op=ALU.add,
                    )
                if ch["assist"] >= 3:
                    g.drain()
                    g.tensor_tensor(
                        bview(ch, 120), bview(ch, 120),
                        bview(ch, 240)[:, :, 120:240], op=ALU.add,
                    )
                if ch["assist"] >= 4:
                    g.drain()
                    g.tensor_tensor(
                        bview(ch, 60), bview(ch, 60),
                        bview(ch, 120)[:, :, 60:120], op=ALU.add,
                    )
                g.drain().then_inc(s_tp, 1)

        # -------------------------------------------------------------- SP --
        @block.sync
        def _(sync):
            for n, i in enumerate(SP_DMA_ORDER):
                ch = info[i]
                if n >= NXF:
                    prev = info[SP_DMA_ORDER[n - NXF]]
                    sync.wait_ge(sems[prev["sq_sem"]], prev["sq_ord"] + 1)
                b0, nb = ch["blk0"], ch["nblk"]
                sync.dma_start(
                    out=xf[ch["xslot"]][:, 0 : nb * BLK],
                    in_=xy[:, b0 * BLK : (b0 + nb) * BLK],
                ).then_inc(s_sp, 16)
            sync.wait_ge(s_tv, n_trees + 1)
            sync.dma_start(out=out, in_=zavear).then_inc(s_out, 16)
            sync.wait_ge(s_out, 16)

        # ------------------------------------------------------------- ACT --
        @block.scalar
        def _(scalar):
            for n, i in enumerate(ACT_SQ_ORDER):
                ch = info[i]
                nb = ch["nblk"]
                if ch["kind"] == "H":
                    scalar.wait_ge(s_pl, 16 * (ch["dma_ord"] + 1))
                    src = xh[ch["xslot"]]
                else:
                    scalar.wait_ge(s_sp, 16 * (ch["dma_ord"] + 1))
                    src = xf[ch["xslot"]]
                if ch["kind"] == "F" and ch["fsq_ord"] >= NSQA:
                    prev = info[F_SQA[ch["fsq_ord"] - NSQA]]
                    scalar.wait_ge(s_tv, prev["tree_ord"] + 1)
                scalar.activation(
                    sq_tile(ch)[:, 0 : nb * BLK], src[:, 0 : nb * BLK], AF.Square
                )
                scalar.drain().then_inc(s_qa, 1)


        # ------------------------------------------------------------- DVE --
        @block.vector
        def _(vector):
            ALU_ = ALU

            def tree_waits(ch):
                if ch["assist"]:
                    vector.wait_ge(s_tp, ch["as_ord"] + 1)
                    return {1: 480, 2: 240, 3: 120, 4: 60}[ch["assist"]]
                if ch["sq_sem"] == "qa":
                    vector.wait_ge(s_qa, ch["sq_ord"] + 1)
                elif ch["sq_sem"] == "qp":
                    vector.wait_ge(s_qp, ch["sq_ord"] + 1)
                return 960

            def emit_trees(group):
                # interleave the chunks' in-place fold chains so one drain
                # covers a whole stage across the group (same-engine RAW is
                # per-chunk; different chunks touch disjoint tiles)
                ws = [tree_waits(c) for c in group]
                while any(w > 60 for w in ws):
                    for k, c in enumerate(group):
                        if ws[k] > 60:
                            h = ws[k] // 2
                            vector.tensor_tensor(
                                bview(c, h), bview(c, h),
                                bview(c, ws[k])[:, :, h : ws[k]], op=ALU_.add,
                            )
                            ws[k] = h
                    vector.drain()
                for c in group:
                    vector.tensor_reduce(
                        bs_slice(c), bview(c, 60), axis=AX.X, op=ALU_.add
                    )
                vector.drain().then_inc(s_tv, len(group))

            def pairable(a, b):
                # only pair late trees whose data is already banked; early
                # pairing couples waits and stalls upstream producers
                return (
                    READY[a["id"]] > 120.0
                    and READY[b["id"]] > 120.0
                    and abs(READY[a["id"]] - READY[b["id"]]) < 8.0
                )

            pending = None
            for job, i in DVE_ORDER:
                ch = info[i]
                nb = ch["nblk"]
                if job == "sq":
                    if pending is not None:
                        emit_trees([pending])
                        pending = None
                    vector.wait_ge(s_pl, 16 * (ch["dma_ord"] + 1))
                    src = xh[ch["xslot"]][:, 0 : nb * BLK]
                    vector.tensor_tensor(
                        sq_tile(ch)[:, 0 : nb * BLK], src, src, op=ALU_.mult
                    )
                    vector.drain().then_inc(s_qd, 1)
                    continue
                if pending is not None:
                    if pairable(pending, ch):
                        emit_trees([pending, ch])
                        pending = None
                    else:
                        emit_trees([pending])
                        pending = ch
                else:
                    pending = ch
            if pending is not None:
                emit_trees([pending])

            # -------------------------------------------------- epilogue ----
            base = bs[:, 0:1]
            frames_view = type(base)(
                tensor=base.tensor,
                offset=base.offset,
                ap=[list(base.ap[0]), [3, NFRM], [1, FRAME // BLK]],
            )
            vector.tensor_reduce(zs, frames_view, axis=AX.X, op=ALU_.add)
            vector.drain()
            vector.scalar_tensor_tensor(
                out=ga, in0=zs, scalar=TA_ZS, in1=zs, op0=ALU_.is_gt, op1=ALU_.mult
            )
            vector.tensor_scalar(ma, zs, TA_ZS, None, op0=ALU_.is_gt)
            vector.drain()
            vector.reduce_sum(numa, ga, axis=AX.X)
            vector.reduce_sum(dena, ma, axis=AX.X)
            vector.drain()
            vector.tensor_scalar_add(dena, dena, EPS)
            vector.drain()
            vector.reciprocal(rca, dena)
            vector.drain()
            vector.tensor_tensor(zavea, numa, rca, op=ALU_.mult)
            vector.drain()
            vector.tensor_scalar(thr, zavea, 0.1, TR_OFF_ZS, op0=ALU_.mult, op1=ALU_.add)
            vector.drain()
            vector.scalar_tensor_tensor(
                out=gar, in0=zs, scalar=thr, in1=ma, op0=ALU_.is_gt, op1=ALU_.mult
            )
            vector.scalar_tensor_tensor(
                out=ga, in0=zs, scalar=thr, in1=ga, op0=ALU_.is_gt, op1=ALU_.mult
            )
            vector.drain()
            vector.reduce_sum(denar, gar, axis=AX.X)
            vector.reduce_sum(numar, ga, axis=AX.X)
            vector.drain()
            vector.tensor_scalar_add(denar, denar, EPS)
            vector.drain()
            vector.reciprocal(rcar, denar)
            vector.drain()
            vector.tensor_tensor(zavear, numar, rcar, op=ALU_.mult)
            vector.drain().then_inc(s_tv, 1)

    return nc


def make_in_maps(x_env: np.ndarray, y_env: np.ndarray) -> list[dict[str, np.ndarray]]:
    x = np.asarray(x_env, dtype=np.float32).reshape(ROWS, T)
    y = np.asarray(y_env, dtype=np.float32).reshape(ROWS, T)
    in_maps = []
    for i in range(N_CORES):
        shard = np.concatenate(
            [x[i * RPC : (i + 1) * RPC], y[i * RPC : (i + 1) * RPC]], axis=0
        )
        in_maps.append({"xy": np.ascontiguousarray(shard)})
    return in_maps


def finish(per_core_zavear: list[np.ndarray]) -> np.ndarray:
    total = 0.0
    for za in per_core_zavear:
        za = np.asarray(za).reshape(128).astype(np.float64)
        lf = C_LUFS + LN10_INV10 * np.log(za + EPS_LN)
        total += np.maximum(lf[RPC:] - lf[:RPC], 0.0).sum()
    return np.array(ALPHA * total, dtype=np.float32)


def kernel(x_env: np.ndarray, y_env: np.ndarray) -> np.ndarray:
    nc = _build_program()
    in_maps = make_in_maps(x_env, y_env)
    res = run_bass_kernel_spmd(nc, in_maps, core_ids=list(range(N_CORES)))
    return finish([res.results[i]["lufs"] for i in range(N_CORES)])


# revision 16
# speedup vs baseline: 1.0246x; 1.0006x over previous
"""DHASPI level-loss kernel v2 for 8 Trainium2 NeuronCores.

Data-parallel over the fused B*C row axis (64 x-rows + 64 y-rows per core in
the 128 SBUF partitions). The work is spread across all four engine queues:

- Pool (gpsimd SWDGE): casting DMAs f32 HBM -> fp16 SBUF (half the modeled
  DMA cost) for "H" chunks, plus in-place fold-assists on late chunks.
- SP (sync HWDGE): f32 DMAs for "F" chunks, running concurrently with Pool's.
- ACT: squares (f32 or fp16 in -> fp16 out) + the final Ln.
- DVE: fp16 squares of some H chunks (2x perf mode), block-sum fold trees
  (in-place halving adds at 2x + a final 60-wide tensor_reduce), epilogue.

Block sums: each 960-sample block is folded in place inside its square tile
960->480->240->120->60, then one tensor_reduce produces the per-block sums.
All bulk tiles are fp16 (rel err ~5e-4, far inside the 2e-2 gate); the gated
loudness math is f32 with the 1/FRAME scaling folded into the log constants.

The last 960 samples of each row feed no analysis frame and are never loaded.
"""

import math

import numpy as np

import concourse.bass as bass
from concourse import mybir
from concourse.bass_utils import run_bass_kernel_spmd

B, C, T = 16, 32, 192000
N_CORES = 8
ROWS = B * C
RPC = ROWS // N_CORES

FRAME = 9600
SHIFT = 2880
BLK = 960
NBLK_USED = 199          # block 199 (samples 191040..192000) feeds no frame
NFRM = (T - FRAME) // SHIFT + 1  # 64

EPS = 1e-8
ALPHA = 1e-4
GAMMA_A = -70.0
# zs = 9600 * z domain constants
TA_ZS = float(FRAME * (10.0 ** ((GAMMA_A + 0.691) / 10.0) - EPS))
TR_OFF_ZS = float(-0.9 * FRAME * EPS)
EPS_LN = float(FRAME * EPS)
LN10_INV10 = float(10.0 / math.log(10.0))
C_LUFS = float(-0.691 - 10.0 * math.log10(FRAME))

F32 = mybir.dt.float32
F16 = mybir.dt.float16

# ---------------------------------------------------------------- schedule --
# Chunk = contiguous run of 960-sample blocks (all 128 rows).
# kind 'H': fp16 via Pool cast-DMA | 'F': f32 via SP DMA
# sq 'A': ACT square | 'D': DVE square (H only)
# assist: 0 = DVE folds alone; 2/3 = Pool folds in place to 240/120 first


def _schedule():
    chunks = []

    def add(kind, nblk, sq, assist=0):
        chunks.append(
            {"kind": kind, "nblk": nblk, "sq": sq, "assist": assist, "id": len(chunks)}
        )

    # warm-up slices (alternating DVE/ACT squares) so compute starts early
    for k in range(5):
        add("H", 2, "D" if k % 2 == 0 else "A")
    for k in range(14):         # h1..h14; every 3rd is Pool-squared
        add("H", 5, "P" if k % 3 == 2 else "A")
    # late H chunks at 4800, Pool fold-assists to 120-wide; the last 3 are
    # fully Pool-owned (square+folds back-to-back on Pool's queue)
    for k in range(8):
        add("H", 5, "D", assist=4 if k == 7 else (3 if k >= 6 else 2))
    # F region: remaining 79 blocks: 14 x 4800 + tail 4 x 1920 + 960
    for _ in range(14):
        add("F", 5, "A")
    for _ in range(4):
        add("F", 2, "A")
    add("F", 1, "A")

    off = 0
    for ch in chunks:
        if ch["kind"] == "H":
            ch["blk0"] = off
            off += ch["nblk"]
    for ch in chunks:
        if ch["kind"] == "F":
            ch["blk0"] = off
            off += ch["nblk"]
    assert off == NBLK_USED, off
    return chunks


CHUNKS = _schedule()
H_IDS = [c["id"] for c in CHUNKS if c["kind"] == "H"]
F_IDS = [c["id"] for c in CHUNKS if c["kind"] == "F"]
POOL_DMA_ORDER = H_IDS
SP_DMA_ORDER = F_IDS
ASSIST_IDS = [c["id"] for c in CHUNKS if c["assist"]]

# estimated DMA completion times (us-ish units) for ordering heuristics
_t_pool = 0.0
_t_sp = 0.0
READY = {}
for _ch in CHUNKS:
    if _ch["kind"] == "H":
        _t_pool += 7.402 * _ch["nblk"] / 10.0
        READY[_ch["id"]] = _t_pool + 2.9
    else:
        _t_sp += 14.805 * _ch["nblk"] / 10.0
        READY[_ch["id"]] = _t_sp + 2.8

ACT_SQ_ORDER = sorted(
    (c["id"] for c in CHUNKS if c["sq"] == "A"), key=lambda i: READY[i]
)
# DVE-squared chunks in arrival order
D_SEQ = sorted((c["id"] for c in CHUNKS if c["sq"] == "D"), key=lambda i: READY[i])
D_POS = {i: n for n, i in enumerate(D_SEQ)}


def _dve_order():
    # 'sq' jobs at arrival; assisted tree of D-chunk k goes after the square
    # of D-chunk k+2 so every Pool<->DVE wait points backwards in both queues
    jobs = []
    for ch in CHUNKS:
        i = ch["id"]
        if ch["sq"] == "D":
            jobs.append((READY[i], 0, ("sq", i)))
            t = READY[i] + (6.0 if ch["assist"] else 0.5)
            if ch["assist"]:
                k = D_POS[i]
                if k + 2 < len(D_SEQ):
                    t = max(t, READY[D_SEQ[k + 2]] + 0.2)
                else:
                    t = max(t, READY[D_SEQ[-1]] + 0.2 + 0.01 * k)
            jobs.append((t, 1, ("tree", i)))
        elif ch["sq"] == "P":
            jobs.append((READY[i] + (10.0 if ch["assist"] else 8.5), 1, ("tree", i)))
        else:
            jobs.append((READY[i] + 8.3, 1, ("tree", i)))
    jobs.sort()
    return [j for _, _, j in jobs]


DVE_ORDER = _dve_order()
DVE_SQ_SEQ = [i for j, i in DVE_ORDER if j == "sq"]


def _pool_prog():
    # assist for D-chunk at D_SEQ position k rides right after the DMA of
    # D-chunk k+2 (whose square will wait for this assist's chunk tree);
    # Pool squares ('P') ride one DMA after their own
    after_dma = {}
    tail = []
    for a in ASSIST_IDS:
        if a not in D_POS:
            continue  # P-chunk assists are emitted with their psq below
        k = D_POS[a]
        if k + 2 < len(D_SEQ):
            after_dma.setdefault(D_SEQ[k + 2], []).append(("assist", a))
        else:
            tail.append(("assist", a))
    for n, i in enumerate(H_IDS):
        if CHUNKS[i]["sq"] == "P":
            jobs = [("psq", i)]
            if CHUNKS[i]["assist"]:
                jobs.append(("assist", i))
            if n + 1 < len(H_IDS):
                after_dma.setdefault(H_IDS[n + 1], []).extend(jobs)
            else:
                tail.extend(jobs)
    prog = []
    for i in H_IDS:
        prog.append(("dma", i))
        prog.extend(after_dma.get(i, []))
    prog.extend(tail)
    return prog


POOL_PROG = _pool_prog()

NXH = 6   # fp16 input slots [128, 4800]; H squares+folds run in place here
NXF = 6   # f32 input slots [128, 4800]
NSQA = 3  # fp16 square slots for F chunks [128, 4800]


def _build_program() -> bass.Bass:
    nc = bass.Bass("TRN2", target_bir_lowering=False, debug=False)
    AF = mybir.ActivationFunctionType
    ALU = mybir.AluOpType
    AX = mybir.AxisListType

    xy = nc.dram_tensor("xy", [128, T], F32, kind="ExternalInput").ap()
    out = nc.dram_tensor("lufs", [128, 1], F32, kind="ExternalOutput").ap()

    xh = [nc.alloc_sbuf_tensor(f"xh{i}", [128, FRAME // 2], F16).ap() for i in range(NXH)]
    xf = [nc.alloc_sbuf_tensor(f"xf{i}", [128, FRAME // 2], F32).ap() for i in range(NXF)]
    sqA = [nc.alloc_sbuf_tensor(f"sqA{i}", [128, FRAME // 2], F16).ap() for i in range(NSQA)]
    bs = nc.alloc_sbuf_tensor("bs", [128, 200], F16).ap()
    zs = nc.alloc_sbuf_tensor("zs", [128, NFRM], F32).ap()
    ga = nc.alloc_sbuf_tensor("ga", [128, NFRM], F32).ap()
    ma = nc.alloc_sbuf_tensor("ma", [128, NFRM], F32).ap()
    gar = nc.alloc_sbuf_tensor("gar", [128, NFRM], F32).ap()
    sc = nc.alloc_sbuf_tensor("sc", [128, 12], F32).ap()

    numa, dena, rca, zavea = sc[:, 0:1], sc[:, 1:2], sc[:, 2:3], sc[:, 3:4]
    thr, denar, numar, rcar = sc[:, 4:5], sc[:, 5:6], sc[:, 6:7], sc[:, 7:8]
    zavear, lnz, lufs_t = sc[:, 8:9], sc[:, 9:10], sc[:, 10:11]

    # ---- bookkeeping ----------------------------------------------------
    info = {c["id"]: dict(c) for c in CHUNKS}
    for n, i in enumerate(POOL_DMA_ORDER):
        info[i]["dma_ord"] = n
        info[i]["xslot"] = n % NXH
    for n, i in enumerate(SP_DMA_ORDER):
        info[i]["dma_ord"] = n
        info[i]["xslot"] = n % NXF
    for n, i in enumerate(ACT_SQ_ORDER):
        info[i]["sq_sem"] = "qa"
        info[i]["sq_ord"] = n
    for n, i in enumerate(DVE_SQ_SEQ):
        info[i]["sq_sem"] = "qd"
        info[i]["sq_ord"] = n
    for n, i in enumerate([c["id"] for c in CHUNKS if c["sq"] == "P"]):
        info[i]["sq_sem"] = "qp"
        info[i]["sq_ord"] = n
    F_SQA = [i for i in ACT_SQ_ORDER if info[i]["kind"] == "F"]
    for n, i in enumerate(F_SQA):
        info[i]["sq_slot"] = n % NSQA
        info[i]["fsq_ord"] = n
    _pool_assists = [i for k, i in POOL_PROG if k == "assist"]
    assert sorted(_pool_assists) == sorted(ASSIST_IDS)
    for n, i in enumerate(_pool_assists):
        info[i]["as_ord"] = n
    n = 0
    for job, i in DVE_ORDER:
        if job == "tree":
            info[i]["tree_ord"] = n
            n += 1
    n_trees = n

    def sq_tile(ch):
        # H chunks square and fold in place inside their xh slot
        if ch["kind"] == "H":
            return xh[ch["xslot"]]
        return sqA[ch["sq_slot"]]

    def bs_slice(ch):
        return bs[:, ch["blk0"] : ch["blk0"] + ch["nblk"]]

    def bview(ch, w):
        """[128, nblk, w] view of the chunk's square tile (block stride 960)."""
        t = sq_tile(ch)
        base = t[:, 0:1]
        return type(base)(
            tensor=base.tensor,
            offset=base.offset,
            ap=[list(base.ap[0]), [BLK, ch["nblk"]], [1, w]],
        )

    with (
        nc.Block() as block,
        nc.semaphore("s_pl") as s_pl,
        nc.semaphore("s_sp") as s_sp,
        nc.semaphore("s_qa") as s_qa,
        nc.semaphore("s_qd") as s_qd,
        nc.semaphore("s_tp") as s_tp,
        nc.semaphore("s_qp") as s_qp,
        nc.semaphore("s_tv") as s_tv,
        nc.semaphore("s_ln") as s_ln,
        nc.semaphore("s_out") as s_out,
        nc.allow_low_precision("fp16 block sums; rel err ~5e-4 vs 2e-2 gate"),
    ):
        sems = {"qa": s_qa, "qd": s_qd, "qp": s_qp}

        # ------------------------------------------------------------ Pool --
        @block.gpsimd
        def _(g):
            ndma = 0
            for kind_, i in POOL_PROG:
                ch = info[i]
                nb = ch["nblk"]
                if kind_ == "dma":
                    if ndma >= NXH:
                        prev = info[POOL_DMA_ORDER[ndma - NXH]]
                        g.wait_ge(s_tv, prev["tree_ord"] + 1)
                    b0 = ch["blk0"]
                    g.dma_start(
                        out=xh[ch["xslot"]][:, 0 : nb * BLK],
                        in_=xy[:, b0 * BLK : (b0 + nb) * BLK],
                    ).then_inc(s_pl, 16)
                    ndma += 1
                    continue
                if kind_ == "psq":
                    g.wait_ge(s_pl, 16 * (ch["dma_ord"] + 1))
                    t = xh[ch["xslot"]][:, 0 : nb * BLK]
                    g.tensor_tensor(t, t, t, op=ALU.mult)
                    g.drain().then_inc(s_qp, 1)
                    continue
                # in-place fold assist on the chunk's square tile
                g.wait_ge(sems[ch["sq_sem"]], ch["sq_ord"] + 1)
                g.tensor_tensor(
                    bview(ch, 480), bview(ch, 480),
                    bview(ch, 960)[:, :, 480:960], op=ALU.add,
                )
                if ch["assist"] >= 2:
                    g.drain()
                    g.tensor_tensor(
                        bview(ch, 240), bview(ch, 240),
                        bview(ch, 480)[:, :, 240:480], op=ALU.add,
                    )
                if ch["assist"] >= 3:
                    g.drain()
                    g.tensor_tensor(
                        bview(ch, 120), bview(ch, 120),
                        bview(ch, 240)[:, :, 120:240], op=ALU.add,
                    )
                if ch["assist"] >= 4:
                    g.drain()
                    g.tensor_tensor(
                        bview(ch, 60), bview(ch, 60),
                        bview(ch, 120)[:, :, 60:120], op=ALU.add,
                    )
                g.drain().then_inc(s_tp, 1)

        # -------------------------------------------------------------- SP --
        @block.sync
        def _(sync):
            for n, i in enumerate(SP_DMA_ORDER):
                ch = info[i]
                if n >= NXF:
                    prev = info[SP_DMA_ORDER[n - NXF]]
                    sync.wait_ge(sems[prev["sq_sem"]], prev["sq_ord"] + 1)
                b0, nb = ch["blk0"], ch["nblk"]
                sync.dma_start(
                    out=xf[ch["xslot"]][:, 0 : nb * BLK],
                    in_=xy[:, b0 * BLK : (b0 + nb) * BLK],
                ).then_inc(s_sp, 16)
            sync.wait_ge(s_tv, n_trees + 1)
            sync.dma_start(out=out, in_=zavear).then_inc(s_out, 16)
            sync.wait_ge(s_out, 16)

        # ------------------------------------------------------------- ACT --
        @block.scalar
        def _(scalar):
            for n, i in enumerate(ACT_SQ_ORDER):
                ch = info[i]
                nb = ch["nblk"]
                if ch["kind"] == "H":
                    scalar.wait_ge(s_pl, 16 * (ch["dma_ord"] + 1))
                    src = xh[ch["xslot"]]
                else:
                    scalar.wait_ge(s_sp, 16 * (ch["dma_ord"] + 1))
                    src = xf[ch["xslot"]]
                if ch["kind"] == "F" and ch["fsq_ord"] >= NSQA:
                    prev = info[F_SQA[ch["fsq_ord"] - NSQA]]
                    scalar.wait_ge(s_tv, prev["tree_ord"] + 1)
                scalar.activation(
                    sq_tile(ch)[:, 0 : nb * BLK], src[:, 0 : nb * BLK], AF.Square
                )
                scalar.drain().then_inc(s_qa, 1)


        # ------------------------------------------------------------- DVE --
        @block.vector
        def _(vector):
            ALU_ = ALU

            def tree_waits(ch):
                if ch["assist"]:
                    vector.wait_ge(s_tp, ch["as_ord"] + 1)
                    return {1: 480, 2: 240, 3: 120, 4: 60}[ch["assist"]]
                if ch["sq_sem"] == "qa":
                    vector.wait_ge(s_qa, ch["sq_ord"] + 1)
                elif ch["sq_sem"] == "qp":
                    vector.wait_ge(s_qp, ch["sq_ord"] + 1)
                return 960

            def emit_trees(group):
                # interleave the chunks' in-place fold chains so one drain
                # covers a whole stage across the group (same-engine RAW is
                # per-chunk; different chunks touch disjoint tiles)
                ws = [tree_waits(c) for c in group]
                while any(w > 60 for w in ws):
                    for k, c in enumerate(group):
                        if ws[k] > 60:
                            h = ws[k] // 2
                            vector.tensor_tensor(
                                bview(c, h), bview(c, h),
                                bview(c, ws[k])[:, :, h : ws[k]], op=ALU_.add,
                            )
                            ws[k] = h
                    vector.drain()
                for c in group:
                    vector.tensor_reduce(
                        bs_slice(c), bview(c, 60), axis=AX.X, op=ALU_.add
                    )
                vector.drain().then_inc(s_tv, len(group))

            def pairable(a, b):
                # only pair late trees whose data is already banked; early
                # pairing couples waits and stalls upstream producers
                return (
                    READY[a["id"]] > 120.0
                    and READY[b["id"]] > 120.0
                    and abs(READY[a["id"]] - READY[b["id"]]) < 8.0
                )

            pending = None
            for job, i in DVE_ORDER:
                ch = info[i]
                nb = ch["nblk"]
                if job == "sq":
                    if pending is not None:
                        emit_trees([pending])
                        pending = None
                    vector.wait_ge(s_pl, 16 * (ch["dma_ord"] + 1))
                    src = xh[ch["xslot"]][:, 0 : nb * BLK]
                    vector.tensor_tensor(
                        sq_tile(ch)[:, 0 : nb * BLK], src, src, op=ALU_.mult
                    )
                    vector.drain().then_inc(s_qd, 1)
                    continue
                if pending is not None:
                    if pairable(pending, ch):
                        emit_trees([pending, ch])
                        pending = None
                    else:
                        emit_trees([pending])
                        pending = ch
                else:
                    pending = ch
            if pending is not None:
                emit_trees([pending])

            # -------------------------------------------------- epilogue ----
            base = bs[:, 0:1]
            frames_view = type(base)(
                tensor=base.tensor,
                offset=base.offset,
                ap=[list(base.ap[0]), [3, NFRM], [1, FRAME // BLK]],
            )
            vector.tensor_reduce(zs, frames_view, axis=AX.X, op=ALU_.add)
            vector.drain()
            vector.scalar_tensor_tensor(
                out=ga, in0=zs, scalar=TA_ZS, in1=zs, op0=ALU_.is_gt, op1=ALU_.mult
            )
            vector.tensor_scalar(ma, zs, TA_ZS, None, op0=ALU_.is_gt)
            vector.drain()
            vector.reduce_sum(numa, ga, axis=AX.X)
            vector.reduce_sum(dena, ma, axis=AX.X)
            vector.drain()
            vector.tensor_scalar_add(dena, dena, EPS)
            vector.drain()
            vector.reciprocal(rca, dena)
            vector.drain()
            vector.scalar_tensor_tensor(
                out=thr, in0=numa, scalar=0.1, in1=rca, op0=ALU_.mult, op1=ALU_.mult
            )
            vector.drain()
            vector.scalar_tensor_tensor(
                out=gar, in0=zs, scalar=thr, in1=ma, op0=ALU_.is_gt, op1=ALU_.mult
            )
            vector.scalar_tensor_tensor(
                out=ga, in0=zs, scalar=thr, in1=ga, op0=ALU_.is_gt, op1=ALU_.mult
            )
            vector.drain()
            vector.reduce_sum(denar, gar, axis=AX.X)
            vector.reduce_sum(numar, ga, axis=AX.X)
            vector.drain()
            vector.tensor_scalar_add(denar, denar, EPS)
            vector.drain()
            vector.reciprocal(rcar, denar)
            vector.drain()
            vector.tensor_tensor(zavear, numar, rcar, op=ALU_.mult)
            vector.drain().then_inc(s_tv, 1)

    return nc


def make_in_maps(x_env: np.ndarray, y_env: np.ndarray) -> list[dict[str, np.ndarray]]:
    x = np.asarray(x_env, dtype=np.float32).reshape(ROWS, T)
    y = np.asarray(y_env, dtype=np.float32).reshape(ROWS, T)
    in_maps = []
    for i in range(N_CORES):
        shard = np.concatenate(
            [x[i * RPC : (i + 1) * RPC], y[i * RPC : (i + 1) * RPC]], axis=0
        )
        in_maps.append({"xy": np.ascontiguousarray(shard)})
    return in_maps


def finish(per_core_zavear: list[np.ndarray]) -> np.ndarray:
    total = 0.0
    for za in per_core_zavear:
        za = np.asarray(za).reshape(128).astype(np.float64)
        lf = C_LUFS + LN10_INV10 * np.log(za + EPS_LN)
        total += np.maximum(lf[RPC:] - lf[:RPC], 0.0).sum()
    return np.array(ALPHA * total, dtype=np.float32)


def kernel(x_env: np.ndarray, y_env: np.ndarray) -> np.ndarray:
    nc = _build_program()
    in_maps = make_in_maps(x_env, y_env)
    res = run_bass_kernel_spmd(nc, in_maps, core_ids=list(range(N_CORES)))
    return finish([res.results[i]["lufs"] for i in range(N_CORES)])
